# revision 1
# baseline (speedup 1.0000x reference)
"""Block-sparse MoE (dense expert-parallel) Trainium2 kernel.

Problem: nn_BlockSparseMoE_15882789061249
  T=1024 tokens, H=2048 hidden, F=1408 intermediate, E=16 experts, top_k=6.

Strategy (8 NeuronCores, SPMD single program):
  - Expert parallel: core c owns experts {2c, 2c+1}. wv1/w2 sharded by
    expert on the host; x and the gate are replicated (x is 8 MB vs 554 MB
    of weights, so replicating x beats an all-to-all token dispatch at this
    scale).
  - Host permutes the gate columns per core so that each core's own two
    experts land in route columns 0 and 1 -> a single SPMD program works
    for every core (top-k mask / renormalization are permutation-invariant).
  - On-core: fp32 router (logits -> exp -> top-6 via DVE max8/match_replace
    -> renormalized weights), bf16 expert matmuls (weights pre-cast and
    pre-tiled on host), SiLU on ScalarE, per-token combine via per-partition
    scalar multiply, DMA-accumulate of the two local experts into a DRAM
    partial, then an 8-core ReduceScatter; each core emits its 128-token
    output shard and the host concatenates shards.
"""

import numpy as np

T, H, F, E = 1024, 2048, 1408, 16
NCORES = 8
TOPK = 6

_CACHE = {}


def build_moe_nc(t, h, f, e, n_cores, topk=6):
    """Build + compile the SPMD Bass program for one core (same for all)."""
    import concourse.bacc as bacc
    import concourse.mybir as mybir
    import concourse.tile as tile

    f32 = mybir.dt.float32
    bf16 = mybir.dt.bfloat16
    AF = mybir.ActivationFunctionType
    Alu = mybir.AluOpType
    X = mybir.AxisListType.X

    epc = e // n_cores          # experts per core
    kh = h // 128               # contraction tiles over hidden
    kf = f // 128               # contraction tiles over intermediate
    mt = t // 128               # token tiles
    mf2 = 2 * f // 128          # fused gate+up row tiles
    tsh = t // n_cores          # output shard tokens
    nt = [(i, min(512, t - i)) for i in range(0, t, 512)]
    nh = [(i, min(512, h - i)) for i in range(0, h, 512)]

    nc = bacc.Bacc("TRN2", target_bir_lowering=False, debug=False,
                   num_devices=n_cores)

    xT = nc.dram_tensor("xT", [h, t], f32, kind="ExternalInput")
    xbT = nc.dram_tensor("xbT", [h, t], bf16, kind="ExternalInput")
    gwT = nc.dram_tensor("gwT", [h, e], f32, kind="ExternalInput")
    wv1t = nc.dram_tensor("wv1t", [epc, kh, mf2, 128, 128], bf16,
                          kind="ExternalInput")
    w2t = nc.dram_tensor("w2t", [epc, kf, 128, h], bf16, kind="ExternalInput")
    ident = nc.dram_tensor("ident", [128, 128], f32, kind="ExternalInput")
    out_sh = nc.dram_tensor("out_shard", [tsh, h], bf16,
                            kind="ExternalOutput")

    # partial + collective run in bf16: halves accumulate-DMA and
    # reduce-scatter traffic; adds ~0.3% absmax error (budget is 2e-2)
    partial = nc.dram_tensor("partial", [t, h], bf16)
    rs_out = nc.dram_tensor("rs_out", [tsh, h], bf16)

    with tile.TileContext(nc) as tc:
        with tc.tile_pool(name="persist", bufs=1) as pp:
            xb = pp.tile([128, kh * t], bf16, tag="xb")
            gw = pp.tile([128, kh * e], f32, tag="gw")
            ids = pp.tile([128, 128], f32, tag="ids")
            route = pp.tile([128, mt * e], f32, tag="route")
            act = pp.tile([128, epc * kf * t], bf16, tag="act")
            lg = pp.tile([128, t], f32, tag="lg")

            nc.sync.dma_start(out=ids[:], in_=ident[:, :])
            for k in range(kh):
                nc.sync.dma_start(out=gw[:, k * e:(k + 1) * e],
                                  in_=gwT[k * 128:(k + 1) * 128, :])

            # bf16 x comes pre-cast from the host so phase A's k-tiles are
            # ready at DMA pace, independent of the fp32 router path
            for k in range(kh):
                nc.sync.dma_start(out=xb[:, k * t:(k + 1) * t],
                                  in_=xbT[k * 128:(k + 1) * 128, :])

            # ---- load x (fp32), router logits [e, t] ----
            with (tc.tile_pool(name="xload", bufs=6) as pxl,
                  tc.tile_pool(name="psr", bufs=1, space="PSUM") as ppr):
                psl = ppr.tile([128, t], f32, tag="psl")
                for k in range(kh):
                    xf = pxl.tile([128, t], f32, tag="xf")
                    nc.sync.dma_start(out=xf[:],
                                      in_=xT[k * 128:(k + 1) * 128, :])
                    for (n0, nsz) in nt:
                        nc.tensor.matmul(
                            psl[:e, n0:n0 + nsz],
                            lhsT=gw[:, k * e:(k + 1) * e],
                            rhs=xf[:, n0:n0 + nsz],
                            start=(k == 0), stop=(k == kh - 1))
                nc.vector.tensor_copy(out=lg[:e, :], in_=psl[:e, :])

            # ---- router: per token tile, top-k renormalized weights ----
            with (tc.tile_pool(name="rt", bufs=2) as prt,
                  tc.tile_pool(name="pst", bufs=2, space="PSUM") as ppt):
                for tt in range(mt):
                    ptile = ppt.tile([128, e], f32, tag="ltr")
                    nc.tensor.transpose(ptile[:, :],
                                        lg[:e, tt * 128:(tt + 1) * 128],
                                        ids[:e, :e])
                    mx = prt.tile([128, 1], f32, tag="mx")
                    nc.vector.reduce_max(out=mx[:], in_=ptile[:, :], axis=X)
                    nm = prt.tile([128, 1], f32, tag="nm")
                    nc.vector.tensor_scalar_mul(nm[:], mx[:], -1.0)
                    ev = prt.tile([128, e], f32, tag="ev")
                    nc.scalar.activation(ev[:], ptile[:, :], AF.Exp,
                                         bias=nm[:], scale=1.0)
                    t8 = prt.tile([128, 8], f32, tag="t8")
                    nc.vector.max(out=t8[:], in_=ev[:])
                    if topk < 8:
                        nc.vector.memset(t8[:, topk:], 0.0)
                    zap = prt.tile([128, e], f32, tag="zap")
                    nc.vector.match_replace(out=zap[:], in_to_replace=t8[:],
                                            in_values=ev[:], imm_value=0.0)
                    msk = prt.tile([128, e], f32, tag="msk")
                    nc.vector.tensor_sub(msk[:], ev[:], zap[:])
                    dn = prt.tile([128, 1], f32, tag="dn")
                    nc.vector.reduce_sum(out=dn[:], in_=msk[:], axis=X)
                    iv = prt.tile([128, 1], f32, tag="iv")
                    nc.vector.reciprocal(iv[:], dn[:])
                    nc.vector.tensor_scalar_mul(
                        route[:, tt * e:(tt + 1) * e], msk[:], iv[:])

            # ---- phase A: act[f, t] = silu(g) * u per local expert ----
            # The first few (le, m) tiles run in a 4-bank PSUM scope that
            # coexists with the router's 2 banks, so expert matmuls fill
            # the PE while the router still runs; the rest use the full
            # 8-bank double-buffered scope.
            def emit_a(le, m, pool, pwv, psg):
                pg = pool.tile([128, t], f32, tag="pg")
                pu = pool.tile([128, t], f32, tag="pu")
                for k in range(kh):
                    wg = pwv.tile([128, 128], bf16, tag="wg")
                    nc.sync.dma_start(out=wg[:], in_=wv1t[le, k, m])
                    wu = pwv.tile([128, 128], bf16, tag="wu")
                    nc.sync.dma_start(out=wu[:], in_=wv1t[le, k, m + kf])
                    # one weight load serves both N-halves
                    for (n0, nsz) in nt:
                        rh = xb[:, k * t + n0:k * t + n0 + nsz]
                        nc.tensor.matmul(pg[:, n0:n0 + nsz],
                                         lhsT=wg[:], rhs=rh,
                                         start=(k == 0), stop=(k == kh - 1))
                    for (n0, nsz) in nt:
                        rh = xb[:, k * t + n0:k * t + n0 + nsz]
                        nc.tensor.matmul(pu[:, n0:n0 + nsz],
                                         lhsT=wu[:], rhs=rh,
                                         start=(k == 0), stop=(k == kh - 1))
                sgm = psg.tile([128, t], bf16, tag="sgm")
                nc.scalar.activation(sgm[:], pg[:], AF.Sigmoid)
                sg = psg.tile([128, t], bf16, tag="sg")
                nc.vector.tensor_mul(out=sg[:], in0=sgm[:], in1=pg[:])
                ai = (le * kf + m) * t
                nc.vector.tensor_mul(out=act[:, ai:ai + t],
                                     in0=sg[:], in1=pu[:])

            pairs = [(le, m) for le in range(epc) for m in range(kf)]
            # with host-cast xb the early tiles' inputs are ready ~1us in,
            # so the 4-bank early scope can genuinely overlap the router
            n_early = min(3, len(pairs))
            with (tc.tile_pool(name="wv", bufs=16) as pwv,
                  tc.tile_pool(name="sg", bufs=3) as psg):
                with tc.tile_pool(name="psaE", bufs=1,
                                  space="PSUM") as ppae:
                    for (le, m) in pairs[:n_early]:
                        emit_a(le, m, ppae, pwv, psg)
                with tc.tile_pool(name="psa", bufs=2, space="PSUM") as ppa:
                    for (le, m) in pairs[n_early:]:
                        emit_a(le, m, ppa, pwv, psg)

            # ---- phase B: y = act @ w2T, combine with route weights ----
            with (tc.tile_pool(name="w2p", bufs=kf + 3) as pw2,
                  tc.tile_pool(name="sc", bufs=3) as psc,
                  tc.tile_pool(name="psb", bufs=2, space="PSUM") as ppb):
                for le in range(epc):
                    w2ks = []
                    for k in range(kf):
                        w2k = pw2.tile([128, h], bf16, tag="w2k")
                        nc.sync.dma_start(out=w2k[:], in_=w2t[le, k])
                        w2ks.append(w2k)
                    for tt in range(mt):
                        py = ppb.tile([128, h], f32, tag="py")
                        for k in range(kf):
                            ai = (le * kf + k) * t + tt * 128
                            for (n0, nsz) in nh:
                                nc.tensor.matmul(
                                    py[:, n0:n0 + nsz],
                                    lhsT=act[:, ai:ai + 128],
                                    rhs=w2ks[k][:, n0:n0 + nsz],
                                    start=(k == 0), stop=(k == kf - 1))
                        rcol = route[:, tt * e + le:tt * e + le + 1]
                        sc = psc.tile([128, h], bf16, tag="sc")
                        nc.vector.tensor_scalar_mul(sc[:], py[:, :], rcol)
                        dst = partial[tt * 128:(tt + 1) * 128, :]
                        if le == 0:
                            nc.sync.dma_start(out=dst, in_=sc[:])
                        else:
                            nc.gpsimd.dma_start(out=dst, in_=sc[:],
                                                accum_op=Alu.add)

            # ---- cross-core reduce-scatter + shard output (fp32 out) ----
            nc.gpsimd.collective_compute(
                "ReduceScatter", Alu.add,
                replica_groups=[list(range(n_cores))],
                ins=[partial.ap().opt()],
                outs=[rs_out.ap().opt()],
            )
            # shards stay bf16; the host casts to fp32 on reassembly
            nc.sync.dma_start(out=out_sh[:, :], in_=rs_out[:, :])

    nc.compile()
    return nc


def prep_inputs(x, gate_w, wv1, w2, t, h, f, e, n_cores):
    """Host-side shard/cast/tile. Returns per-core input maps."""
    import ml_dtypes
    bf16 = ml_dtypes.bfloat16

    epc = e // n_cores
    kh = h // 128
    kf = f // 128
    mf2 = 2 * f // 128

    xT = np.ascontiguousarray(x.T).astype(np.float32)        # [h, t]
    xbT = xT.astype(bf16)                                    # [h, t] bf16
    ident = np.eye(128, dtype=np.float32)

    in_maps = []
    for c in range(n_cores):
        own = list(range(c * epc, (c + 1) * epc))
        rest = [i for i in range(e) if i not in own]
        perm = own + rest
        gwT = np.ascontiguousarray(gate_w[perm].T).astype(np.float32)

        wl = wv1[own]                                        # [epc, 2f, h]
        # wv1t[le, k, m, hp, fp] = wv1[own[le], m*128+fp, k*128+hp]
        wv1tc = np.ascontiguousarray(
            wl.transpose(0, 2, 1)                            # [epc, h, 2f]
              .reshape(epc, kh, 128, mf2, 128)
              .transpose(0, 1, 3, 2, 4)).astype(bf16)

        w2l = w2[own]                                        # [epc, h, f]
        # w2t[le, k, fp, hh] = w2[own[le], hh, k*128+fp]
        w2tc = np.ascontiguousarray(
            w2l.transpose(0, 2, 1)                           # [epc, f, h]
               .reshape(epc, kf, 128, h)).astype(bf16)

        in_maps.append({
            "xT": xT,
            "xbT": xbT,
            "gwT": gwT,
            "wv1t": wv1tc,
            "w2t": w2tc,
            "ident": ident,
        })
    return in_maps


def unshard(shards, t, h, n_cores):
    """Reassemble the full output from per-core RS shards (rank order)."""
    return np.concatenate(shards, axis=0).astype(np.float32)


def kernel(x, gate_w, wv1, w2, top_k):
    from concourse.bass_utils import run_bass_kernel_spmd

    assert int(top_k) == TOPK
    x = np.asarray(x, dtype=np.float32)
    gate_w = np.asarray(gate_w, dtype=np.float32)
    wv1 = np.asarray(wv1, dtype=np.float32)
    w2 = np.asarray(w2, dtype=np.float32)

    key = (T, H, F, E, NCORES)
    if key not in _CACHE:
        _CACHE[key] = build_moe_nc(T, H, F, E, NCORES, TOPK)
    nc = _CACHE[key]

    in_maps = prep_inputs(x, gate_w, wv1, w2, T, H, F, E, NCORES)
    res = run_bass_kernel_spmd(nc, in_maps, list(range(NCORES)))
    shards = [res.results[c]["out_shard"] for c in range(NCORES)]
    return unshard(shards, T, H, NCORES)



# revision 10
# speedup vs baseline: 2.1043x; 2.1043x over previous
"""Block-sparse MoE (sparse expert-parallel dispatch) Trainium2 kernel.

Problem: nn_BlockSparseMoE_15882789061249
  T=1024 tokens, H=2048 hidden, F=1408 intermediate, E=16 experts, top_k=6.

Strategy (8 NeuronCores, SPMD single program):
  - Expert parallel: core c owns experts {2c, 2c+1}; wv1/w2 sharded by
    expert on the host, gate replicated (columns permuted per core so the
    own experts land in route columns 0/1 -> one SPMD program).
  - Sparse dispatch: only top_k=6 of 16 experts contribute per token, so
    each expert needs only ~6/16 of the tokens. The host computes the
    routing *metadata* (which tokens each expert needs, with a 1e-4
    relative margin around the 6th prob so host/device top-k can never
    disagree) and ships per-expert gathered token matrices of capacity
    C=512 (actual max count is 418). All *numerics* stay on device: the
    fp32 router (logits -> softmax -> top-6 -> renorm), the expert MLPs
    on the gathered tokens, the route-weight combine, and the cross-core
    reduce-scatter.
  - Slots are bucketed by token-tile *pair* (4 buckets x 128 slots per
    expert; max actual bucket is 112), which makes the scatter-back
    pattern compile-time static: slot-chunk j only touches token tiles
    2j/2j+1. Scatter-back is a matmul with a host-provided 0/1 selection
    matrix, weighted on-device by the routed probabilities.
  - Weights are laid out so every DMA line is 2-4 KiB contiguous (the
    old per-[128,128]-tile layout moved 256 B lines and throttled the
    PE array to ~60% in phase A).
  - The reduce-scatter runs in 4 chunks of 2 token tiles, each fired as
    soon as its partial is complete, hiding most of the collective
    behind compute. Each core emits 4x [32, 2048] shards; the host
    reassembles them.
"""

import numpy as np

T, H, F, E = 1024, 2048, 1408, 16
NCORES = 8
TOPK = 6
EPC = E // NCORES            # experts per core (2)
C = 512                      # gathered-token capacity per expert
NB = 4                       # slot buckets per expert (token-tile pairs)
BK = C // NB                 # slots per bucket (128)
KH = H // 128                # 16
KF = F // 128                # 11
MF2 = 2 * F // 128           # 22
MT = T // 128                # 8 token tiles
MARGIN = 1e-4                # relative margin on the 6th prob

_CACHE = {}


def build_moe_nc(n_cores=NCORES):
    """Build + compile the SPMD Bass program for one core (same for all)."""
    import concourse.bacc as bacc
    import concourse.mybir as mybir
    import concourse.tile as tile

    f32 = mybir.dt.float32
    bf16 = mybir.dt.bfloat16
    AF = mybir.ActivationFunctionType
    Alu = mybir.AluOpType
    X = mybir.AxisListType.X

    t, e = T, E
    nc = bacc.Bacc("TRN2", target_bir_lowering=False, debug=False,
                   num_devices=n_cores)

    xT = nc.dram_tensor("xT", [H, t], f32, kind="ExternalInput")
    gwp = nc.dram_tensor("gwp", [128, KH * e], f32, kind="ExternalInput")
    xgd = nc.dram_tensor("xgd", [KH, 128, EPC * C], bf16,
                         kind="ExternalInput")
    seld = nc.dram_tensor("seld", [EPC, NB, 128, t], bf16,
                          kind="ExternalInput")
    wgd = nc.dram_tensor("wgd", [EPC, MF2, 128, KH * 128], bf16,
                         kind="ExternalInput")
    w2d = nc.dram_tensor("w2d", [EPC, KF, 128, H], bf16,
                         kind="ExternalInput")

    shw = 2 * 128 // n_cores
    parts = [nc.dram_tensor(f"partial{j}", [2 * 128, H], bf16)
             for j in range(NB)]
    rss = [nc.dram_tensor(f"rsi{j}", [shw, H], bf16) for j in range(NB)]
    out_sh = nc.dram_tensor("out_sh", [NB * shw, H], bf16,
                            kind="ExternalOutput")

    W2PRE = 4                # e0 w2 k-tiles prefetched at program start

    with tile.TileContext(nc) as tc:
        with tc.tile_pool(name="persist", bufs=1) as pp:
            gw = pp.tile([128, KH * e], f32, tag="gw")
            lg = pp.tile([128, t], f32, tag="lg")
            route = pp.tile([128, MT * e + 32], f32, tag="route")
            ltr = pp.tile([128, MT * 32], f32, tag="ltr")
            rqs = pp.tile([128, EPC * t], f32, tag="rqs")
            rbc = pp.tile([128, EPC * t], f32, tag="rbc")
            act = pp.tile([128, EPC * KF * C], bf16, tag="act")
            sels = pp.tile([128, EPC * NB * t], bf16, tag="sels")
            selw = pp.tile([128, EPC * NB * t], bf16, tag="selw")
            w2p0 = pp.tile([128, W2PRE * H], bf16, tag="w2p0")

            nc.sync.dma_start(out=gw[:], in_=gwp[:, :])
            for le in range(EPC):
                for jj in range(NB):
                    blk = (le * NB + jj) * t
                    nc.sync.dma_start(out=sels[:, blk:blk + t],
                                      in_=seld[le, jj])
            for kk in range(W2PRE):
                nc.sync.dma_start(out=w2p0[:, kk * H:(kk + 1) * H],
                                  in_=w2d[0, kk])

            # rows 16:32 of lg feed the padded 32x32 transposes below; the
            # copy from psl overwrites rows :16 afterwards (32-aligned base)
            nc.vector.memset(lg[0:32, :], 0.0)
            nc.vector.memset(route[:, MT * e:], 0.0)
            nc.vector.memset(rqs[0:32, :], 0.0)

            with (tc.tile_pool(name="xg", bufs=1) as pxg,
                  tc.tile_pool(name="wv", bufs=4) as pwv,
                  tc.tile_pool(name="xf", bufs=3) as pxf,
                  tc.tile_pool(name="sg", bufs=3) as psg,
                  tc.tile_pool(name="rt", bufs=2) as prt,
                  tc.tile_pool(name="psa", bufs=3, space="PSUM") as ppa):
                xg = pxg.tile([128, KH * EPC * C], bf16, tag="xg")
                for k in range(KH):
                    nc.sync.dma_start(
                        out=xg[:, k * EPC * C:(k + 1) * EPC * C],
                        in_=xgd[k])

                # ---- router logits [e, t] (fp32) ----
                with tc.tile_pool(name="psr", bufs=1,
                                  space="PSUM") as ppr:
                    psl = ppr.tile([128, t], f32, tag="psl")
                    for k in range(KH):
                        xf = pxf.tile([128, t], f32, tag="xf")
                        nc.sync.dma_start(out=xf[:],
                                          in_=xT[k * 128:(k + 1) * 128, :])
                        for n0 in range(0, t, 512):
                            nc.tensor.matmul(
                                psl[:e, n0:n0 + 512],
                                lhsT=gw[:, k * e:(k + 1) * e],
                                rhs=xf[:, n0:n0 + 512],
                                start=(k == 0), stop=(k == KH - 1))
                    nc.vector.tensor_copy(out=lg[:e, :], in_=psl[:e, :])

                # ---- top-k per token tile (all DVE; PE stays on MLPs) ----
                for tt in range(MT):
                    for b in range(4):
                        nc.vector.transpose(
                            out=ltr[b * 32:(b + 1) * 32,
                                    tt * 32:(tt + 1) * 32],
                            in_=lg[0:32, tt * 128 + b * 32:
                                   tt * 128 + (b + 1) * 32])
                    ev_in = ltr[:, tt * 32:tt * 32 + e]
                    mx = prt.tile([128, 1], f32, tag="mx")
                    nc.vector.reduce_max(out=mx[:], in_=ev_in, axis=X)
                    nm = prt.tile([128, 1], f32, tag="nm")
                    nc.vector.tensor_scalar_mul(nm[:], mx[:], -1.0)
                    ev = prt.tile([128, e], f32, tag="ev")
                    nc.scalar.activation(ev[:], ev_in, AF.Exp,
                                         bias=nm[:], scale=1.0)
                    t8 = prt.tile([128, 8], f32, tag="t8")
                    nc.vector.max(out=t8[:], in_=ev[:])
                    nc.vector.memset(t8[:, TOPK:], 0.0)
                    zap = prt.tile([128, e], f32, tag="zap")
                    nc.vector.match_replace(out=zap[:], in_to_replace=t8[:],
                                            in_values=ev[:], imm_value=0.0)
                    msk = prt.tile([128, e], f32, tag="msk")
                    nc.vector.tensor_sub(msk[:], ev[:], zap[:])
                    dn = prt.tile([128, 1], f32, tag="dn")
                    nc.vector.reduce_sum(out=dn[:], in_=msk[:], axis=X)
                    iv = prt.tile([128, 1], f32, tag="iv")
                    nc.vector.reciprocal(iv[:], dn[:])
                    nc.vector.tensor_scalar_mul(
                        route[:, tt * e:(tt + 1) * e], msk[:], iv[:])

                # route -> [expert, token] layout. The transpose input is
                # shifted by `le` so own-expert column le lands on partition
                # 0 of its block (compute APs need 32-aligned partition
                # bases, so reading rqs[1:2, :] later would be illegal).
                for le in range(EPC):
                    for tt in range(MT):
                        for b in range(4):
                            nc.vector.transpose(
                                out=rqs[0:32, le * t + tt * 128 + b * 32:
                                        le * t + tt * 128 + (b + 1) * 32],
                                in_=route[b * 32:(b + 1) * 32,
                                          tt * e + le:tt * e + le + 32])
                for le in range(EPC):
                    nc.gpsimd.partition_broadcast(
                        rbc[:, le * t:(le + 1) * t],
                        rqs[0:1, le * t:le * t + t])
                for le in range(EPC):
                    for jj in range(NB):
                        blk = (le * NB + jj) * t
                        nc.vector.tensor_mul(
                            out=selw[:, blk:blk + t],
                            in0=sels[:, blk:blk + t],
                            in1=rbc[:, le * t:(le + 1) * t])

                # ---- phase A: act[f, slots] = silu(g) * u per expert ----
                for le in range(EPC):
                    for mm in range(KF):
                        wg = pwv.tile([128, KH * 128], bf16, tag="wg")
                        nc.sync.dma_start(out=wg[:], in_=wgd[le, mm])
                        wu = pwv.tile([128, KH * 128], bf16, tag="wu")
                        nc.sync.dma_start(out=wu[:], in_=wgd[le, KF + mm])
                        pg = ppa.tile([128, C], f32, tag="pg")
                        pu = ppa.tile([128, C], f32, tag="pu")
                        for k in range(KH):
                            rh = xg[:, k * EPC * C + le * C:
                                    k * EPC * C + (le + 1) * C]
                            nc.tensor.matmul(pg[:],
                                             lhsT=wg[:, k * 128:(k + 1) * 128],
                                             rhs=rh,
                                             start=(k == 0),
                                             stop=(k == KH - 1))
                        for k in range(KH):
                            rh = xg[:, k * EPC * C + le * C:
                                    k * EPC * C + (le + 1) * C]
                            nc.tensor.matmul(pu[:],
                                             lhsT=wu[:, k * 128:(k + 1) * 128],
                                             rhs=rh,
                                             start=(k == 0),
                                             stop=(k == KH - 1))
                        sgm = psg.tile([128, C], bf16, tag="sgm")
                        nc.scalar.activation(sgm[:], pg[:], AF.Sigmoid)
                        sg = psg.tile([128, C], bf16, tag="sg")
                        nc.vector.tensor_mul(out=sg[:], in0=sgm[:], in1=pg[:])
                        ai = (le * KF + mm) * C
                        nc.vector.tensor_mul(out=act[:, ai:ai + C],
                                             in0=sg[:], in1=pu[:])

            # ---- phase B + weighted scatter + chunked reduce-scatter ----
            with (tc.tile_pool(name="w2p", bufs=EPC * KF - W2PRE) as pw2,
                  tc.tile_pool(name="yb", bufs=4) as pyb,
                  tc.tile_pool(name="so", bufs=2) as pso,
                  tc.tile_pool(name="psb", bufs=3, space="PSUM") as ppb,
                  tc.tile_pool(name="psc", bufs=2, space="PSUM") as ppc):
                w2sb = {}
                for le in range(EPC):
                    for kk in range(KF):
                        if le == 0 and kk < W2PRE:
                            w2sb[(le, kk)] = w2p0[:, kk * H:(kk + 1) * H]
                        else:
                            w2k = pw2.tile([128, H], bf16, tag="w2k")
                            nc.sync.dma_start(out=w2k[:], in_=w2d[le, kk])
                            w2sb[(le, kk)] = w2k[:]

                for jj in range(NB):
                    ybs = {}
                    for le in range(EPC):
                        py = [ppb.tile([128, 1024], f32, tag="py",
                                       name=f"py{jj}_{le}_{hh}")
                              for hh in range(2)]
                        for kk in range(KF):
                            lh = act[:, (le * KF + kk) * C + jj * BK:
                                     (le * KF + kk) * C + (jj + 1) * BK]
                            w2t_ = w2sb[(le, kk)]
                            for hh in range(2):
                                for q in range(2):
                                    n0 = q * 512
                                    nc.tensor.matmul(
                                        py[hh][:, n0:n0 + 512],
                                        lhsT=lh,
                                        rhs=w2t_[:, hh * 1024 + n0:
                                                 hh * 1024 + n0 + 512],
                                        start=(kk == 0), stop=(kk == KF - 1))
                        yb = pyb.tile([128, H], bf16, tag="yb")
                        for hh in range(2):
                            nc.vector.tensor_copy(
                                out=yb[:, hh * 1024:(hh + 1) * 1024],
                                in_=py[hh][:])
                        ybs[le] = yb

                    for tt in (2 * jj, 2 * jj + 1):
                        so = pso.tile([128, H], bf16, tag="so")
                        for hq in range(4):
                            ps = ppc.tile([128, 512], f32, tag="ps")
                            for le in range(EPC):
                                blk = (le * NB + jj) * t
                                nc.tensor.matmul(
                                    ps[:],
                                    lhsT=selw[:, blk + tt * 128:
                                              blk + (tt + 1) * 128],
                                    rhs=ybs[le][:, hq * 512:(hq + 1) * 512],
                                    start=(le == 0), stop=(le == EPC - 1))
                            nc.vector.tensor_copy(
                                out=so[:, hq * 512:(hq + 1) * 512],
                                in_=ps[:])
                        nc.sync.dma_start(
                            out=parts[jj][(tt % 2) * 128:
                                          (tt % 2 + 1) * 128, :],
                            in_=so[:])

                    nc.gpsimd.collective_compute(
                        "ReduceScatter", Alu.add,
                        replica_groups=[list(range(n_cores))],
                        ins=[parts[jj].ap().opt()],
                        outs=[rss[jj].ap().opt()],
                    )
                    nc.sync.dma_start(
                        out=out_sh[jj * shw:(jj + 1) * shw, :],
                        in_=rss[jj][:, :])

    nc.compile()
    return nc


def _route_sel(x, gate_w):
    """Host routing metadata: top-6 membership with a tie margin."""
    lg = x.astype(np.float64) @ gate_w.astype(np.float64).T
    lg -= lg.max(axis=1, keepdims=True)
    p = np.exp(lg)
    p /= p.sum(axis=1, keepdims=True)
    sp = -np.sort(-p, axis=1)
    thr = sp[:, TOPK - 1:TOPK] * (1.0 - MARGIN)
    return p >= thr


def prep_inputs(x, gate_w, wv1, w2, *_unused):
    """Host-side shard/gather/cast/tile. Returns per-core input maps."""
    import ml_dtypes
    bf16 = ml_dtypes.bfloat16

    x = np.asarray(x, dtype=np.float32)
    gate_w = np.asarray(gate_w, dtype=np.float32)
    sel = _route_sel(x, gate_w)                       # [T, E] bool
    tp = np.arange(T) // (2 * 128)                    # token-tile pair id

    xTf = np.ascontiguousarray(x.T).astype(np.float32)

    in_maps = []
    for c in range(NCORES):
        own = list(range(c * EPC, (c + 1) * EPC))
        rest = [i for i in range(E) if i not in own]
        perm = own + rest
        gp = gate_w[perm].T.astype(np.float32)        # [H, E]
        gwp = np.ascontiguousarray(
            gp.reshape(KH, 128, E).transpose(1, 0, 2).reshape(128, KH * E))

        toks = np.full((EPC, NB, BK), -1, dtype=np.int64)
        for le, ee in enumerate(own):
            for jj in range(NB):
                tt = np.nonzero(sel[:, ee] & (tp == jj))[0]
                if len(tt) > BK:
                    raise ValueError(
                        f"bucket overflow: expert {ee} pair {jj}: {len(tt)}")
                toks[le, jj, :len(tt)] = tt
        valid = toks >= 0
        tok0 = np.where(valid, toks, 0)

        xs = x[tok0.reshape(-1)].reshape(EPC, C, H) \
            * valid.reshape(EPC, C, 1)
        xgd = np.ascontiguousarray(
            xs.reshape(EPC, C, KH, 128).transpose(2, 3, 0, 1)
              .reshape(KH, 128, EPC * C)).astype(bf16)

        seldf = np.zeros((EPC, NB, BK, T), dtype=np.float32)
        il, ij, ii = np.nonzero(valid)
        seldf[il, ij, ii, toks[valid]] = 1.0
        seld = seldf.astype(bf16)

        wl = wv1[own]                                 # [EPC, 2F, H]
        wgd = np.ascontiguousarray(
            wl.reshape(EPC, MF2, 128, KH, 128)
              .transpose(0, 1, 4, 3, 2)
              .reshape(EPC, MF2, 128, KH * 128)).astype(bf16)

        w2l = w2[own]                                 # [EPC, H, F]
        w2d = np.ascontiguousarray(
            w2l.transpose(0, 2, 1).reshape(EPC, KF, 128, H)).astype(bf16)

        in_maps.append({
            "xT": xTf,
            "gwp": gwp,
            "xgd": xgd,
            "seld": seld,
            "wgd": wgd,
            "w2d": w2d,
        })
    return in_maps


def unshard(per_core_results):
    """Reassemble [T, H] from each core's stacked rs chunks."""
    shw = 2 * 128 // NCORES                           # 32 rows per chunk
    out = np.empty((T, H), dtype=np.float32)
    for c, res in enumerate(per_core_results):
        sh = np.asarray(res["out_sh"]).astype(np.float32)
        for jj in range(NB):
            base = jj * 2 * 128 + c * shw
            out[base:base + shw, :] = sh[jj * shw:(jj + 1) * shw, :]
    return out


def kernel(x, gate_w, wv1, w2, top_k):
    from concourse.bass_utils import run_bass_kernel_spmd

    assert int(top_k) == TOPK
    x = np.asarray(x, dtype=np.float32)
    gate_w = np.asarray(gate_w, dtype=np.float32)
    wv1 = np.asarray(wv1, dtype=np.float32)
    w2 = np.asarray(w2, dtype=np.float32)

    key = (T, H, F, E, NCORES, C)
    if key not in _CACHE:
        _CACHE[key] = build_moe_nc(NCORES)
    nc = _CACHE[key]

    in_maps = prep_inputs(x, gate_w, wv1, w2)
    res = run_bass_kernel_spmd(nc, in_maps, list(range(NCORES)))
    return unshard([res.results[c] for c in range(NCORES)])


# revision 14
# speedup vs baseline: 2.1897x; 1.0405x over previous
"""Block-sparse MoE (sparse expert-parallel dispatch) Trainium2 kernel.

Problem: nn_BlockSparseMoE_15882789061249
  T=1024 tokens, H=2048 hidden, F=1408 intermediate, E=16 experts, top_k=6.

Strategy (8 NeuronCores, SPMD single program):
  - Expert parallel: core c owns experts {2c, 2c+1}; wv1/w2 sharded by
    expert on the host, gate replicated (columns permuted per core so the
    own experts land in route columns 0/1 -> one SPMD program).
  - Sparse dispatch: only top_k=6 of 16 experts contribute per token, so
    each expert needs only ~6/16 of the tokens. The host computes the
    routing *metadata* (which tokens each expert needs, with a 1e-4
    relative margin around the 6th prob so host/device top-k can never
    disagree) and ships per-expert gathered token matrices of capacity
    C=512 (actual max count is 418). All *numerics* stay on device: the
    fp32 router (logits -> softmax -> top-6 -> renorm), the expert MLPs
    on the gathered tokens, the route-weight combine, and the cross-core
    reduce-scatter.
  - Slots are bucketed by token-tile *pair* (4 buckets x 128 slots per
    expert; max actual bucket is 112), which makes the scatter-back
    pattern compile-time static: slot-chunk j only touches token tiles
    2j/2j+1. Scatter-back is a matmul with a host-provided 0/1 selection
    matrix, weighted on-device by the routed probabilities.
  - Weights are laid out so every DMA line is 2-4 KiB contiguous (the
    old per-[128,128]-tile layout moved 256 B lines and throttled the
    PE array to ~60% in phase A).
  - The reduce-scatter runs in 4 chunks of 2 token tiles, each fired as
    soon as its partial is complete, hiding most of the collective
    behind compute. Each core emits 4x [32, 2048] shards; the host
    reassembles them.
"""

import numpy as np

T, H, F, E = 1024, 2048, 1408, 16
NCORES = 8
TOPK = 6
EPC = E // NCORES            # experts per core (2)
C = 512                      # gathered-token capacity per expert
NB = 4                       # slot buckets per expert (token-tile pairs)
BK = C // NB                 # slots per bucket (128)
KH = H // 128                # 16
KF = F // 128                # 11
MF2 = 2 * F // 128           # 22
MT = T // 128                # 8 token tiles
MARGIN = 1e-4                # relative margin on the 6th prob

_CACHE = {}


def build_moe_nc(n_cores=NCORES):
    """Build + compile the SPMD Bass program for one core (same for all)."""
    import concourse.bacc as bacc
    import concourse.mybir as mybir
    import concourse.tile as tile

    f32 = mybir.dt.float32
    bf16 = mybir.dt.bfloat16
    AF = mybir.ActivationFunctionType
    Alu = mybir.AluOpType
    X = mybir.AxisListType.X

    t, e = T, E
    nc = bacc.Bacc("TRN2", target_bir_lowering=False, debug=False,
                   num_devices=n_cores)

    xT = nc.dram_tensor("xT", [H, t], f32, kind="ExternalInput")
    gwp = nc.dram_tensor("gwp", [128, KH * e], f32, kind="ExternalInput")
    xgd = nc.dram_tensor("xgd", [KH, 128, EPC * C], bf16,
                         kind="ExternalInput")
    seld = nc.dram_tensor("seld", [EPC, NB, 128, t], bf16,
                          kind="ExternalInput")
    wgd = nc.dram_tensor("wgd", [EPC, MF2, 128, KH * 128], bf16,
                         kind="ExternalInput")
    w2d = nc.dram_tensor("w2d", [EPC, KF, 128, H], bf16,
                         kind="ExternalInput")

    shw = 2 * 128 // n_cores
    parts = [nc.dram_tensor(f"partial{j}", [2 * 128, H], bf16)
             for j in range(NB)]
    rss = [nc.dram_tensor(f"rsi{j}", [shw, H], bf16) for j in range(NB)]
    out_sh = nc.dram_tensor("out_sh", [NB * shw, H], bf16,
                            kind="ExternalOutput")
    wrm_i = nc.dram_tensor("wrm_i", [8, 256], bf16)
    wrm_o = nc.dram_tensor("wrm_o", [1, 256], bf16)

    W2PRE = 4                # e0 w2 k-tiles prefetched at program start

    with tile.TileContext(nc) as tc:
        with tc.tile_pool(name="persist", bufs=1) as pp:
            gw = pp.tile([128, KH * e], f32, tag="gw")
            lg = pp.tile([128, t], f32, tag="lg")
            route = pp.tile([128, MT * e + 32], f32, tag="route")
            ltr = pp.tile([128, MT * 32], f32, tag="ltr")
            rqs = pp.tile([128, EPC * t], f32, tag="rqs")
            rbc = pp.tile([128, EPC * t], f32, tag="rbc")
            act = pp.tile([128, EPC * KF * C], bf16, tag="act")
            sels = pp.tile([128, EPC * NB * t], bf16, tag="sels")
            selw = pp.tile([128, EPC * NB * t], bf16, tag="selw")
            w2p0 = pp.tile([128, W2PRE * H], bf16, tag="w2p0")

            nc.sync.dma_start(out=gw[:], in_=gwp[:, :])

            # rows 16:32 of lg feed the padded 32x32 transposes below; the
            # copy from psl overwrites rows :16 afterwards (32-aligned base)
            nc.vector.memset(lg[0:32, :], 0.0)
            nc.vector.memset(route[:, MT * e:], 0.0)
            nc.vector.memset(rqs[0:32, :], 0.0)

            # tiny collective up front absorbs the cold-start cost of the
            # CC path so the first real reduce-scatter runs at ring speed
            nc.gpsimd.collective_compute(
                "ReduceScatter", Alu.add,
                replica_groups=[list(range(n_cores))],
                ins=[wrm_i.ap().opt()],
                outs=[wrm_o.ap().opt()],
            )

            with (tc.tile_pool(name="xg", bufs=1) as pxg,
                  tc.tile_pool(name="wv", bufs=4) as pwv,
                  tc.tile_pool(name="xf", bufs=3) as pxf,
                  tc.tile_pool(name="sg", bufs=3) as psg,
                  tc.tile_pool(name="rt", bufs=2) as prt,
                  tc.tile_pool(name="psa", bufs=3, space="PSUM") as ppa):
                # first expert-pair weights go ahead of everything else in
                # the DMA queues so the PE can start within a few us
                wg0 = pwv.tile([128, KH * 128], bf16, tag="wg")
                nc.sync.dma_start(out=wg0[:], in_=wgd[0, 0])
                wu0 = pwv.tile([128, KH * 128], bf16, tag="wu")
                nc.sync.dma_start(out=wu0[:], in_=wgd[0, KF])

                xg = pxg.tile([128, KH * EPC * C], bf16, tag="xg")
                for k in range(KH):
                    nc.sync.dma_start(
                        out=xg[:, k * EPC * C:(k + 1) * EPC * C],
                        in_=xgd[k])
                for le in range(EPC):
                    for jj in range(NB):
                        blk = (le * NB + jj) * t
                        nc.sync.dma_start(out=sels[:, blk:blk + t],
                                          in_=seld[le, jj])

                def emit_a(le, mm, wgt, wut):
                    pg = ppa.tile([128, C], f32, tag="pg", name=f"pg{le}_{mm}")
                    pu = ppa.tile([128, C], f32, tag="pu", name=f"pu{le}_{mm}")
                    for k in range(KH):
                        rh = xg[:, k * EPC * C + le * C:
                                k * EPC * C + (le + 1) * C]
                        nc.tensor.matmul(pg[:],
                                         lhsT=wgt[:, k * 128:(k + 1) * 128],
                                         rhs=rh,
                                         start=(k == 0), stop=(k == KH - 1))
                    for k in range(KH):
                        rh = xg[:, k * EPC * C + le * C:
                                k * EPC * C + (le + 1) * C]
                        nc.tensor.matmul(pu[:],
                                         lhsT=wut[:, k * 128:(k + 1) * 128],
                                         rhs=rh,
                                         start=(k == 0), stop=(k == KH - 1))
                    sgm = psg.tile([128, C], bf16, tag="sgm",
                                   name=f"sgm{le}_{mm}")
                    nc.scalar.activation(sgm[:], pg[:], AF.Sigmoid)
                    sg = psg.tile([128, C], bf16, tag="sg",
                                  name=f"sg{le}_{mm}")
                    nc.vector.tensor_mul(out=sg[:], in0=sgm[:], in1=pg[:])
                    ai = (le * KF + mm) * C
                    nc.vector.tensor_mul(out=act[:, ai:ai + C],
                                         in0=sg[:], in1=pu[:])

                def emit_a_range(pairs):
                    for le, mm in pairs:
                        if (le, mm) == (0, 0):
                            emit_a(le, mm, wg0, wu0)
                            continue
                        wg = pwv.tile([128, KH * 128], bf16, tag="wg",
                                      name=f"wg{le}_{mm}")
                        nc.sync.dma_start(out=wg[:], in_=wgd[le, mm])
                        wu = pwv.tile([128, KH * 128], bf16, tag="wu",
                                      name=f"wu{le}_{mm}")
                        nc.sync.dma_start(out=wu[:], in_=wgd[le, KF + mm])
                        emit_a(le, mm, wg, wu)

                all_pairs = [(le, mm) for le in range(EPC)
                             for mm in range(KF)]
                # phase A, first slice: keeps the PE warm while the fp32
                # router matmuls (below) slot into the middle of the stream
                emit_a_range(all_pairs[:14])

                # ---- router logits [e, t] (fp32) ----
                with tc.tile_pool(name="psr", bufs=1,
                                  space="PSUM") as ppr:
                    psl = ppr.tile([128, t], f32, tag="psl")
                    for k in range(KH):
                        xf = pxf.tile([128, t], f32, tag="xf",
                                      name=f"xf{k}")
                        nc.sync.dma_start(out=xf[:],
                                          in_=xT[k * 128:(k + 1) * 128, :])
                        for n0 in range(0, t, 512):
                            nc.tensor.matmul(
                                psl[:e, n0:n0 + 512],
                                lhsT=gw[:, k * e:(k + 1) * e],
                                rhs=xf[:, n0:n0 + 512],
                                start=(k == 0), stop=(k == KH - 1))
                    nc.vector.tensor_copy(out=lg[:e, :], in_=psl[:e, :])

                # ---- top-k per token tile (all DVE; PE stays on MLPs) ----
                for tt in range(MT):
                    for b in range(4):
                        nc.vector.transpose(
                            out=ltr[b * 32:(b + 1) * 32,
                                    tt * 32:(tt + 1) * 32],
                            in_=lg[0:32, tt * 128 + b * 32:
                                   tt * 128 + (b + 1) * 32])
                    ev_in = ltr[:, tt * 32:tt * 32 + e]
                    mx = prt.tile([128, 1], f32, tag="mx")
                    nc.vector.reduce_max(out=mx[:], in_=ev_in, axis=X)
                    nm = prt.tile([128, 1], f32, tag="nm")
                    nc.vector.tensor_scalar_mul(nm[:], mx[:], -1.0)
                    ev = prt.tile([128, e], f32, tag="ev")
                    nc.scalar.activation(ev[:], ev_in, AF.Exp,
                                         bias=nm[:], scale=1.0)
                    t8 = prt.tile([128, 8], f32, tag="t8")
                    nc.vector.max(out=t8[:], in_=ev[:])
                    nc.vector.memset(t8[:, TOPK:], 0.0)
                    zap = prt.tile([128, e], f32, tag="zap")
                    nc.vector.match_replace(out=zap[:], in_to_replace=t8[:],
                                            in_values=ev[:], imm_value=0.0)
                    msk = prt.tile([128, e], f32, tag="msk")
                    nc.vector.tensor_sub(msk[:], ev[:], zap[:])
                    dn = prt.tile([128, 1], f32, tag="dn")
                    nc.vector.reduce_sum(out=dn[:], in_=msk[:], axis=X)
                    iv = prt.tile([128, 1], f32, tag="iv")
                    nc.vector.reciprocal(iv[:], dn[:])
                    nc.vector.tensor_scalar_mul(
                        route[:, tt * e:(tt + 1) * e], msk[:], iv[:])

                # route -> [expert, token] layout. The transpose input is
                # shifted by `le` so own-expert column le lands on partition
                # 0 of its block (compute APs need 32-aligned partition
                # bases, so reading rqs[1:2, :] later would be illegal).
                for le in range(EPC):
                    for tt in range(MT):
                        for b in range(4):
                            nc.vector.transpose(
                                out=rqs[0:32, le * t + tt * 128 + b * 32:
                                        le * t + tt * 128 + (b + 1) * 32],
                                in_=route[b * 32:(b + 1) * 32,
                                          tt * e + le:tt * e + le + 32])
                for le in range(EPC):
                    nc.gpsimd.partition_broadcast(
                        rbc[:, le * t:(le + 1) * t],
                        rqs[0:1, le * t:le * t + t])
                for le in range(EPC):
                    for jj in range(NB):
                        blk = (le * NB + jj) * t
                        nc.vector.tensor_mul(
                            out=selw[:, blk:blk + t],
                            in0=sels[:, blk:blk + t],
                            in1=rbc[:, le * t:(le + 1) * t])

                # ---- phase A, second slice ----
                emit_a_range(all_pairs[14:])

                # w2 prefetch for the first expert's first k-tiles; emitted
                # late so it never crowds the phase-A weight stream
                for kk in range(W2PRE):
                    nc.sync.dma_start(out=w2p0[:, kk * H:(kk + 1) * H],
                                      in_=w2d[0, kk])

            # ---- phase B + weighted scatter + chunked reduce-scatter ----
            with (tc.tile_pool(name="w2p", bufs=EPC * KF - W2PRE) as pw2,
                  tc.tile_pool(name="yb", bufs=4) as pyb,
                  tc.tile_pool(name="so", bufs=2) as pso,
                  tc.tile_pool(name="psb", bufs=3, space="PSUM") as ppb,
                  tc.tile_pool(name="psc", bufs=2, space="PSUM") as ppc):
                w2sb = {}
                for le in range(EPC):
                    for kk in range(KF):
                        if le == 0 and kk < W2PRE:
                            w2sb[(le, kk)] = w2p0[:, kk * H:(kk + 1) * H]
                        else:
                            w2k = pw2.tile([128, H], bf16, tag="w2k")
                            nc.sync.dma_start(out=w2k[:], in_=w2d[le, kk])
                            w2sb[(le, kk)] = w2k[:]

                for jj in range(NB):
                    ybs = {}
                    for le in range(EPC):
                        py = [ppb.tile([128, 1024], f32, tag="py",
                                       name=f"py{jj}_{le}_{hh}")
                              for hh in range(2)]
                        for kk in range(KF):
                            lh = act[:, (le * KF + kk) * C + jj * BK:
                                     (le * KF + kk) * C + (jj + 1) * BK]
                            w2t_ = w2sb[(le, kk)]
                            for hh in range(2):
                                for q in range(2):
                                    n0 = q * 512
                                    nc.tensor.matmul(
                                        py[hh][:, n0:n0 + 512],
                                        lhsT=lh,
                                        rhs=w2t_[:, hh * 1024 + n0:
                                                 hh * 1024 + n0 + 512],
                                        start=(kk == 0), stop=(kk == KF - 1))
                        yb = pyb.tile([128, H], bf16, tag="yb")
                        for hh in range(2):
                            nc.vector.tensor_copy(
                                out=yb[:, hh * 1024:(hh + 1) * 1024],
                                in_=py[hh][:])
                        ybs[le] = yb

                    for tt in (2 * jj, 2 * jj + 1):
                        so = pso.tile([128, H], bf16, tag="so",
                                      name=f"so{tt}")
                        for hq in range(4):
                            ps = ppc.tile([128, 512], f32, tag="ps",
                                          name=f"ps{tt}_{hq}")
                            for le in range(EPC):
                                blk = (le * NB + jj) * t
                                nc.tensor.matmul(
                                    ps[:],
                                    lhsT=selw[:, blk + tt * 128:
                                              blk + (tt + 1) * 128],
                                    rhs=ybs[le][:, hq * 512:(hq + 1) * 512],
                                    start=(le == 0), stop=(le == EPC - 1))
                            # ScalarE does the PSUM drain so the DVE stays
                            # free for the yb casts
                            nc.scalar.activation(
                                so[:, hq * 512:(hq + 1) * 512], ps[:],
                                AF.Copy)
                        nc.sync.dma_start(
                            out=parts[jj][(tt % 2) * 128:
                                          (tt % 2 + 1) * 128, :],
                            in_=so[:])

                # collectives are emitted after the compute loop: each is
                # data-gated on its partial, so they still overlap B/scatter
                # of later chunks, but no sync edges land inside the PE/DVE
                # streams (in-loop emission measurably stalled both)
                for jj in range(NB):
                    nc.gpsimd.collective_compute(
                        "ReduceScatter", Alu.add,
                        replica_groups=[list(range(n_cores))],
                        ins=[parts[jj].ap().opt()],
                        outs=[rss[jj].ap().opt()],
                    )
                    nc.sync.dma_start(
                        out=out_sh[jj * shw:(jj + 1) * shw, :],
                        in_=rss[jj][:, :])

    nc.compile()
    return nc


def _route_sel(x, gate_w):
    """Host routing metadata: top-6 membership with a tie margin."""
    lg = x.astype(np.float64) @ gate_w.astype(np.float64).T
    lg -= lg.max(axis=1, keepdims=True)
    p = np.exp(lg)
    p /= p.sum(axis=1, keepdims=True)
    sp = -np.sort(-p, axis=1)
    thr = sp[:, TOPK - 1:TOPK] * (1.0 - MARGIN)
    return p >= thr


def prep_inputs(x, gate_w, wv1, w2, *_unused):
    """Host-side shard/gather/cast/tile. Returns per-core input maps."""
    import ml_dtypes
    bf16 = ml_dtypes.bfloat16

    x = np.asarray(x, dtype=np.float32)
    gate_w = np.asarray(gate_w, dtype=np.float32)
    sel = _route_sel(x, gate_w)                       # [T, E] bool
    tp = np.arange(T) // (2 * 128)                    # token-tile pair id

    xTf = np.ascontiguousarray(x.T).astype(np.float32)

    in_maps = []
    for c in range(NCORES):
        own = list(range(c * EPC, (c + 1) * EPC))
        rest = [i for i in range(E) if i not in own]
        perm = own + rest
        gp = gate_w[perm].T.astype(np.float32)        # [H, E]
        gwp = np.ascontiguousarray(
            gp.reshape(KH, 128, E).transpose(1, 0, 2).reshape(128, KH * E))

        toks = np.full((EPC, NB, BK), -1, dtype=np.int64)
        for le, ee in enumerate(own):
            for jj in range(NB):
                tt = np.nonzero(sel[:, ee] & (tp == jj))[0]
                if len(tt) > BK:
                    raise ValueError(
                        f"bucket overflow: expert {ee} pair {jj}: {len(tt)}")
                toks[le, jj, :len(tt)] = tt
        valid = toks >= 0
        tok0 = np.where(valid, toks, 0)

        xs = x[tok0.reshape(-1)].reshape(EPC, C, H) \
            * valid.reshape(EPC, C, 1)
        xgd = np.ascontiguousarray(
            xs.reshape(EPC, C, KH, 128).transpose(2, 3, 0, 1)
              .reshape(KH, 128, EPC * C)).astype(bf16)

        seldf = np.zeros((EPC, NB, BK, T), dtype=np.float32)
        il, ij, ii = np.nonzero(valid)
        seldf[il, ij, ii, toks[valid]] = 1.0
        seld = seldf.astype(bf16)

        wl = wv1[own]                                 # [EPC, 2F, H]
        wgd = np.ascontiguousarray(
            wl.reshape(EPC, MF2, 128, KH, 128)
              .transpose(0, 1, 4, 3, 2)
              .reshape(EPC, MF2, 128, KH * 128)).astype(bf16)

        w2l = w2[own]                                 # [EPC, H, F]
        w2d = np.ascontiguousarray(
            w2l.transpose(0, 2, 1).reshape(EPC, KF, 128, H)).astype(bf16)

        in_maps.append({
            "xT": xTf,
            "gwp": gwp,
            "xgd": xgd,
            "seld": seld,
            "wgd": wgd,
            "w2d": w2d,
        })
    return in_maps


def unshard(per_core_results):
    """Reassemble [T, H] from each core's stacked rs chunks."""
    shw = 2 * 128 // NCORES                           # 32 rows per chunk
    out = np.empty((T, H), dtype=np.float32)
    for c, res in enumerate(per_core_results):
        sh = np.asarray(res["out_sh"]).astype(np.float32)
        for jj in range(NB):
            base = jj * 2 * 128 + c * shw
            out[base:base + shw, :] = sh[jj * shw:(jj + 1) * shw, :]
    return out


def kernel(x, gate_w, wv1, w2, top_k):
    from concourse.bass_utils import run_bass_kernel_spmd

    assert int(top_k) == TOPK
    x = np.asarray(x, dtype=np.float32)
    gate_w = np.asarray(gate_w, dtype=np.float32)
    wv1 = np.asarray(wv1, dtype=np.float32)
    w2 = np.asarray(w2, dtype=np.float32)

    key = (T, H, F, E, NCORES, C)
    if key not in _CACHE:
        _CACHE[key] = build_moe_nc(NCORES)
    nc = _CACHE[key]

    in_maps = prep_inputs(x, gate_w, wv1, w2)
    res = run_bass_kernel_spmd(nc, in_maps, list(range(NCORES)))
    return unshard([res.results[c] for c in range(NCORES)])


# revision 19
# speedup vs baseline: 2.2216x; 1.0146x over previous
"""Block-sparse MoE (sparse expert-parallel dispatch) Trainium2 kernel.

Problem: nn_BlockSparseMoE_15882789061249
  T=1024 tokens, H=2048 hidden, F=1408 intermediate, E=16 experts, top_k=6.

Strategy (8 NeuronCores, SPMD single program):
  - Expert parallel: core c owns experts {2c, 2c+1}; wv1/w2 sharded by
    expert on the host, gate replicated (columns permuted per core so the
    own experts land in route columns 0/1 -> one SPMD program).
  - Sparse dispatch: only top_k=6 of 16 experts contribute per token, so
    each expert needs only ~6/16 of the tokens. The host computes the
    routing *metadata* (which tokens each expert needs, with a 1e-4
    relative margin around the 6th prob so host/device top-k can never
    disagree) and ships per-expert gathered token matrices of capacity
    C=512 (actual max count is 418). All *numerics* stay on device: the
    fp32 router (logits -> softmax -> top-6 -> renorm), the expert MLPs
    on the gathered tokens, the route-weight combine, and the cross-core
    reduce-scatter.
  - Slots are bucketed by token-tile *pair* (4 buckets x 128 slots per
    expert; max actual bucket is 112), which makes the scatter-back
    pattern compile-time static: slot-chunk j only touches token tiles
    2j/2j+1. Scatter-back is a matmul with a host-provided 0/1 selection
    matrix, weighted on-device by the routed probabilities.
  - Weights are laid out so every DMA line is 2-4 KiB contiguous (the
    old per-[128,128]-tile layout moved 256 B lines and throttled the
    PE array to ~60% in phase A).
  - The reduce-scatter runs in 4 chunks of 2 token tiles, each fired as
    soon as its partial is complete, hiding most of the collective
    behind compute. Each core emits 4x [32, 2048] shards; the host
    reassembles them.
"""

import numpy as np

T, H, F, E = 1024, 2048, 1408, 16
NCORES = 8
TOPK = 6
EPC = E // NCORES            # experts per core (2)
C = 512                      # gathered-token capacity per expert
NB = 4                       # slot buckets per expert (token-tile pairs)
BK = C // NB                 # slots per bucket (128)
KH = H // 128                # 16
KF = F // 128                # 11
MF2 = 2 * F // 128           # 22
MT = T // 128                # 8 token tiles
MARGIN = 1e-4                # relative margin on the 6th prob

_CACHE = {}


def build_moe_nc(n_cores=NCORES):
    """Build + compile the SPMD Bass program for one core (same for all)."""
    import concourse.bacc as bacc
    import concourse.mybir as mybir
    import concourse.tile as tile

    f32 = mybir.dt.float32
    bf16 = mybir.dt.bfloat16
    AF = mybir.ActivationFunctionType
    Alu = mybir.AluOpType
    X = mybir.AxisListType.X

    t, e = T, E
    nc = bacc.Bacc("TRN2", target_bir_lowering=False, debug=False,
                   num_devices=n_cores)

    xT = nc.dram_tensor("xT", [H, t], f32, kind="ExternalInput")
    gwp = nc.dram_tensor("gwp", [128, KH * e], f32, kind="ExternalInput")
    xgd = nc.dram_tensor("xgd", [KH, 128, EPC * C], bf16,
                         kind="ExternalInput")
    seld = nc.dram_tensor("seld", [EPC, NB, 128, t], bf16,
                          kind="ExternalInput")
    wgd = nc.dram_tensor("wgd", [EPC, MF2, 128, KH * 128], bf16,
                         kind="ExternalInput")
    w2d = nc.dram_tensor("w2d", [EPC, KF, 128, H], bf16,
                         kind="ExternalInput")

    shw = 2 * 128 // n_cores
    parts = [nc.dram_tensor(f"partial{j}", [2 * 128, H], bf16)
             for j in range(NB)]
    rss = [nc.dram_tensor(f"rsi{j}", [shw, H], bf16) for j in range(NB)]
    out_sh = nc.dram_tensor("out_sh", [NB * shw, H], bf16,
                            kind="ExternalOutput")
    wrm_i = nc.dram_tensor("wrm_i", [8, 256], bf16)
    wrm_o = nc.dram_tensor("wrm_o", [1, 256], bf16)

    W2PRE = 8                # e0 w2 k-tiles prefetched before phase B

    with tile.TileContext(nc) as tc:
        with tc.tile_pool(name="persist", bufs=1) as pp:
            gw = pp.tile([128, KH * e], f32, tag="gw")
            lg = pp.tile([128, t], f32, tag="lg")
            route = pp.tile([128, MT * e + 32], f32, tag="route")
            ltr = pp.tile([128, MT * 32], f32, tag="ltr")
            rqs = pp.tile([128, EPC * t], f32, tag="rqs")
            rbc = pp.tile([128, EPC * t], f32, tag="rbc")
            act = pp.tile([128, EPC * KF * C], bf16, tag="act")
            sels = pp.tile([128, EPC * NB * t], bf16, tag="sels")
            selw = pp.tile([128, EPC * NB * t], bf16, tag="selw")
            w2p0 = pp.tile([128, W2PRE * H], bf16, tag="w2p0")
            ones = pp.tile([128, 128], f32, tag="ones")

            nc.sync.dma_start(out=gw[:], in_=gwp[:, :])
            nc.vector.memset(ones[0:32, :], 1.0)

            # rows 16:32 of lg feed the padded 32x32 transposes below; the
            # copy from psl overwrites rows :16 afterwards (32-aligned base)
            nc.vector.memset(lg[0:32, :], 0.0)
            nc.vector.memset(route[:, MT * e:], 0.0)
            nc.vector.memset(rqs[0:32, :], 0.0)

            # tiny collective up front absorbs the cold-start cost of the
            # CC path so the first real reduce-scatter runs at ring speed
            nc.gpsimd.collective_compute(
                "ReduceScatter", Alu.add,
                replica_groups=[list(range(n_cores))],
                ins=[wrm_i.ap().opt()],
                outs=[wrm_o.ap().opt()],
            )

            with (tc.tile_pool(name="xg", bufs=1) as pxg,
                  tc.tile_pool(name="wv", bufs=4) as pwv,
                  tc.tile_pool(name="xf", bufs=3) as pxf,
                  tc.tile_pool(name="sg", bufs=3) as psg,
                  tc.tile_pool(name="rt", bufs=2) as prt,
                  tc.tile_pool(name="psa", bufs=3, space="PSUM") as ppa):
                # first expert-pair weights go ahead of everything else in
                # the DMA queues so the PE can start within a few us
                wg0 = pwv.tile([128, KH * 128], bf16, tag="wg")
                nc.sync.dma_start(out=wg0[:], in_=wgd[0, 0])
                wu0 = pwv.tile([128, KH * 128], bf16, tag="wu")
                nc.sync.dma_start(out=wu0[:], in_=wgd[0, KF])

                xg = pxg.tile([128, KH * EPC * C], bf16, tag="xg")
                for k in range(KH):
                    nc.sync.dma_start(
                        out=xg[:, k * EPC * C:(k + 1) * EPC * C],
                        in_=xgd[k])
                for le in range(EPC):
                    for jj in range(NB):
                        blk = (le * NB + jj) * t
                        nc.sync.dma_start(out=sels[:, blk:blk + t],
                                          in_=seld[le, jj])

                def emit_a(le, mm, wgt, wut):
                    pg = ppa.tile([128, C], f32, tag="pg", name=f"pg{le}_{mm}")
                    pu = ppa.tile([128, C], f32, tag="pu", name=f"pu{le}_{mm}")
                    for k in range(KH):
                        rh = xg[:, k * EPC * C + le * C:
                                k * EPC * C + (le + 1) * C]
                        nc.tensor.matmul(pg[:],
                                         lhsT=wgt[:, k * 128:(k + 1) * 128],
                                         rhs=rh,
                                         start=(k == 0), stop=(k == KH - 1))
                    for k in range(KH):
                        rh = xg[:, k * EPC * C + le * C:
                                k * EPC * C + (le + 1) * C]
                        nc.tensor.matmul(pu[:],
                                         lhsT=wut[:, k * 128:(k + 1) * 128],
                                         rhs=rh,
                                         start=(k == 0), stop=(k == KH - 1))
                    sgm = psg.tile([128, C], bf16, tag="sgm",
                                   name=f"sgm{le}_{mm}")
                    nc.scalar.activation(sgm[:], pg[:], AF.Sigmoid)
                    sg = psg.tile([128, C], bf16, tag="sg",
                                  name=f"sg{le}_{mm}")
                    nc.vector.tensor_mul(out=sg[:], in0=sgm[:], in1=pg[:])
                    ai = (le * KF + mm) * C
                    nc.vector.tensor_mul(out=act[:, ai:ai + C],
                                         in0=sg[:], in1=pu[:])

                def emit_a_range(pairs):
                    for le, mm in pairs:
                        if (le, mm) == (0, 0):
                            emit_a(le, mm, wg0, wu0)
                            continue
                        wg = pwv.tile([128, KH * 128], bf16, tag="wg",
                                      name=f"wg{le}_{mm}")
                        nc.sync.dma_start(out=wg[:], in_=wgd[le, mm])
                        wu = pwv.tile([128, KH * 128], bf16, tag="wu",
                                      name=f"wu{le}_{mm}")
                        nc.sync.dma_start(out=wu[:], in_=wgd[le, KF + mm])
                        emit_a(le, mm, wg, wu)

                all_pairs = [(le, mm) for le in range(EPC)
                             for mm in range(KF)]
                # phase A, first slice: keeps the PE warm while the fp32
                # router matmuls (below) slot into the middle of the stream
                emit_a_range(all_pairs[:14])

                # ---- router logits [e, t] (fp32) ----
                with tc.tile_pool(name="psr", bufs=1,
                                  space="PSUM") as ppr:
                    psl = ppr.tile([128, t], f32, tag="psl")
                    for k in range(KH):
                        xf = pxf.tile([128, t], f32, tag="xf",
                                      name=f"xf{k}")
                        nc.sync.dma_start(out=xf[:],
                                          in_=xT[k * 128:(k + 1) * 128, :])
                        for n0 in range(0, t, 512):
                            nc.tensor.matmul(
                                psl[:e, n0:n0 + 512],
                                lhsT=gw[:, k * e:(k + 1) * e],
                                rhs=xf[:, n0:n0 + 512],
                                start=(k == 0), stop=(k == KH - 1))
                    nc.vector.tensor_copy(out=lg[:e, :], in_=psl[:e, :])

                # ---- top-k per token tile (all DVE; PE stays on MLPs) ----
                for tt in range(MT):
                    for b in range(4):
                        nc.vector.transpose(
                            out=ltr[b * 32:(b + 1) * 32,
                                    tt * 32:(tt + 1) * 32],
                            in_=lg[0:32, tt * 128 + b * 32:
                                   tt * 128 + (b + 1) * 32])
                    ev_in = ltr[:, tt * 32:tt * 32 + e]
                    mx = prt.tile([128, 1], f32, tag="mx")
                    nc.vector.reduce_max(out=mx[:], in_=ev_in, axis=X)
                    nm = prt.tile([128, 1], f32, tag="nm")
                    nc.vector.tensor_scalar_mul(nm[:], mx[:], -1.0)
                    ev = prt.tile([128, e], f32, tag="ev")
                    nc.scalar.activation(ev[:], ev_in, AF.Exp,
                                         bias=nm[:], scale=1.0)
                    t8 = prt.tile([128, 8], f32, tag="t8")
                    nc.vector.max(out=t8[:], in_=ev[:])
                    nc.vector.memset(t8[:, TOPK:], 0.0)
                    zap = prt.tile([128, e], f32, tag="zap")
                    nc.vector.match_replace(out=zap[:], in_to_replace=t8[:],
                                            in_values=ev[:], imm_value=0.0)
                    msk = prt.tile([128, e], f32, tag="msk")
                    nc.vector.tensor_sub(msk[:], ev[:], zap[:])
                    dn = prt.tile([128, 1], f32, tag="dn")
                    nc.vector.reduce_sum(out=dn[:], in_=msk[:], axis=X)
                    iv = prt.tile([128, 1], f32, tag="iv")
                    nc.vector.reciprocal(iv[:], dn[:])
                    nc.vector.tensor_scalar_mul(
                        route[:, tt * e:(tt + 1) * e], msk[:], iv[:])

                # route -> [expert, token] layout. The transpose input is
                # shifted by `le` so own-expert column le lands on partition
                # 0 of its block (compute APs need 32-aligned partition
                # bases, so reading rqs[1:2, :] later would be illegal).
                for le in range(EPC):
                    for tt in range(MT):
                        for b in range(4):
                            nc.vector.transpose(
                                out=rqs[0:32, le * t + tt * 128 + b * 32:
                                        le * t + tt * 128 + (b + 1) * 32],
                                in_=route[b * 32:(b + 1) * 32,
                                          tt * e + le:tt * e + le + 32])
                # ---- phase A, second slice, with the w2 prefetch DMAs
                # interleaved so they land just before phase B needs them
                emit_a_range(all_pairs[14:17])
                for kk in range(W2PRE):
                    nc.sync.dma_start(out=w2p0[:, kk * H:(kk + 1) * H],
                                      in_=w2d[0, kk])
                emit_a_range(all_pairs[17:])

                # broadcast each own-expert route row across all 128
                # partitions via a PE outer product (ones x row). Keeping
                # this off GpSimd matters: gpsimd ucode ops ahead of the
                # collectives were observed to stall the whole CC chain.
                with tc.tile_pool(name="psb2", bufs=1,
                                  space="PSUM") as ppb2:
                    for le in range(EPC):
                        pbc = ppb2.tile([128, t], f32, tag="pbc",
                                        name=f"pbc{le}")
                        for n0 in range(0, t, 512):
                            nc.tensor.matmul(
                                pbc[:, n0:n0 + 512],
                                lhsT=ones[0:1, :],
                                rhs=rqs[0:1, le * t + n0:le * t + n0 + 512],
                                start=True, stop=True)
                        nc.vector.tensor_copy(
                            out=rbc[:, le * t:(le + 1) * t], in_=pbc[:])
                for le in range(EPC):
                    for jj in range(NB):
                        blk = (le * NB + jj) * t
                        nc.vector.tensor_mul(
                            out=selw[:, blk:blk + t],
                            in0=sels[:, blk:blk + t],
                            in1=rbc[:, le * t:(le + 1) * t])

            # ---- phase B + weighted scatter + chunked reduce-scatter ----
            with (tc.tile_pool(name="w2p", bufs=EPC * KF - W2PRE) as pw2,
                  tc.tile_pool(name="yb", bufs=4) as pyb,
                  tc.tile_pool(name="so", bufs=2) as pso,
                  tc.tile_pool(name="psb", bufs=3, space="PSUM") as ppb,
                  tc.tile_pool(name="psc", bufs=2, space="PSUM") as ppc):
                w2sb = {}
                for le in range(EPC):
                    for kk in range(KF):
                        if le == 0 and kk < W2PRE:
                            w2sb[(le, kk)] = w2p0[:, kk * H:(kk + 1) * H]
                        else:
                            w2k = pw2.tile([128, H], bf16, tag="w2k")
                            nc.sync.dma_start(out=w2k[:], in_=w2d[le, kk])
                            w2sb[(le, kk)] = w2k[:]

                for jj in range(NB):
                    ybs = {}
                    for le in range(EPC):
                        py = [ppb.tile([128, 1024], f32, tag="py",
                                       name=f"py{jj}_{le}_{hh}")
                              for hh in range(2)]
                        for kk in range(KF):
                            lh = act[:, (le * KF + kk) * C + jj * BK:
                                     (le * KF + kk) * C + (jj + 1) * BK]
                            w2t_ = w2sb[(le, kk)]
                            for hh in range(2):
                                for q in range(2):
                                    n0 = q * 512
                                    nc.tensor.matmul(
                                        py[hh][:, n0:n0 + 512],
                                        lhsT=lh,
                                        rhs=w2t_[:, hh * 1024 + n0:
                                                 hh * 1024 + n0 + 512],
                                        start=(kk == 0), stop=(kk == KF - 1))
                        yb = pyb.tile([128, H], bf16, tag="yb")
                        # ScalarE drains py so the DVE stays free for the
                        # scatter copies that gate the partial DMAs
                        for hh in range(2):
                            nc.scalar.activation(
                                yb[:, hh * 1024:(hh + 1) * 1024],
                                py[hh][:], AF.Copy)
                        ybs[le] = yb

                    for tt in (2 * jj, 2 * jj + 1):
                        so = pso.tile([128, H], bf16, tag="so",
                                      name=f"so{tt}")
                        for hq in range(4):
                            ps = ppc.tile([128, 512], f32, tag="ps",
                                          name=f"ps{tt}_{hq}")
                            for le in range(EPC):
                                blk = (le * NB + jj) * t
                                nc.tensor.matmul(
                                    ps[:],
                                    lhsT=selw[:, blk + tt * 128:
                                              blk + (tt + 1) * 128],
                                    rhs=ybs[le][:, hq * 512:(hq + 1) * 512],
                                    start=(le == 0), stop=(le == EPC - 1))
                            nc.vector.tensor_copy(
                                out=so[:, hq * 512:(hq + 1) * 512],
                                in_=ps[:])
                        nc.sync.dma_start(
                            out=parts[jj][(tt % 2) * 128:
                                          (tt % 2 + 1) * 128, :],
                            in_=so[:])

                # collectives are emitted after the compute loop: each is
                # data-gated on its partial, so they still overlap B/scatter
                # of later chunks, but no sync edges land inside the PE/DVE
                # streams (in-loop emission measurably stalled both)
                for jj in range(NB):
                    nc.gpsimd.collective_compute(
                        "ReduceScatter", Alu.add,
                        replica_groups=[list(range(n_cores))],
                        ins=[parts[jj].ap().opt()],
                        outs=[rss[jj].ap().opt()],
                    )
                    nc.sync.dma_start(
                        out=out_sh[jj * shw:(jj + 1) * shw, :],
                        in_=rss[jj][:, :])

    nc.compile()
    return nc


def _route_sel(x, gate_w):
    """Host routing metadata: top-6 membership with a tie margin."""
    lg = x.astype(np.float64) @ gate_w.astype(np.float64).T
    lg -= lg.max(axis=1, keepdims=True)
    p = np.exp(lg)
    p /= p.sum(axis=1, keepdims=True)
    sp = -np.sort(-p, axis=1)
    thr = sp[:, TOPK - 1:TOPK] * (1.0 - MARGIN)
    return p >= thr


def prep_inputs(x, gate_w, wv1, w2, *_unused):
    """Host-side shard/gather/cast/tile. Returns per-core input maps."""
    import ml_dtypes
    bf16 = ml_dtypes.bfloat16

    x = np.asarray(x, dtype=np.float32)
    gate_w = np.asarray(gate_w, dtype=np.float32)
    sel = _route_sel(x, gate_w)                       # [T, E] bool
    tp = np.arange(T) // (2 * 128)                    # token-tile pair id

    xTf = np.ascontiguousarray(x.T).astype(np.float32)

    in_maps = []
    for c in range(NCORES):
        own = list(range(c * EPC, (c + 1) * EPC))
        rest = [i for i in range(E) if i not in own]
        perm = own + rest
        gp = gate_w[perm].T.astype(np.float32)        # [H, E]
        gwp = np.ascontiguousarray(
            gp.reshape(KH, 128, E).transpose(1, 0, 2).reshape(128, KH * E))

        toks = np.full((EPC, NB, BK), -1, dtype=np.int64)
        for le, ee in enumerate(own):
            for jj in range(NB):
                tt = np.nonzero(sel[:, ee] & (tp == jj))[0]
                if len(tt) > BK:
                    raise ValueError(
                        f"bucket overflow: expert {ee} pair {jj}: {len(tt)}")
                toks[le, jj, :len(tt)] = tt
        valid = toks >= 0
        tok0 = np.where(valid, toks, 0)

        xs = x[tok0.reshape(-1)].reshape(EPC, C, H) \
            * valid.reshape(EPC, C, 1)
        xgd = np.ascontiguousarray(
            xs.reshape(EPC, C, KH, 128).transpose(2, 3, 0, 1)
              .reshape(KH, 128, EPC * C)).astype(bf16)

        seldf = np.zeros((EPC, NB, BK, T), dtype=np.float32)
        il, ij, ii = np.nonzero(valid)
        seldf[il, ij, ii, toks[valid]] = 1.0
        seld = seldf.astype(bf16)

        wl = wv1[own]                                 # [EPC, 2F, H]
        wgd = np.ascontiguousarray(
            wl.reshape(EPC, MF2, 128, KH, 128)
              .transpose(0, 1, 4, 3, 2)
              .reshape(EPC, MF2, 128, KH * 128)).astype(bf16)

        w2l = w2[own]                                 # [EPC, H, F]
        w2d = np.ascontiguousarray(
            w2l.transpose(0, 2, 1).reshape(EPC, KF, 128, H)).astype(bf16)

        in_maps.append({
            "xT": xTf,
            "gwp": gwp,
            "xgd": xgd,
            "seld": seld,
            "wgd": wgd,
            "w2d": w2d,
        })
    return in_maps


def unshard(per_core_results):
    """Reassemble [T, H] from each core's stacked rs chunks."""
    shw = 2 * 128 // NCORES                           # 32 rows per chunk
    out = np.empty((T, H), dtype=np.float32)
    for c, res in enumerate(per_core_results):
        sh = np.asarray(res["out_sh"]).astype(np.float32)
        for jj in range(NB):
            base = jj * 2 * 128 + c * shw
            out[base:base + shw, :] = sh[jj * shw:(jj + 1) * shw, :]
    return out


def kernel(x, gate_w, wv1, w2, top_k):
    from concourse.bass_utils import run_bass_kernel_spmd

    assert int(top_k) == TOPK
    x = np.asarray(x, dtype=np.float32)
    gate_w = np.asarray(gate_w, dtype=np.float32)
    wv1 = np.asarray(wv1, dtype=np.float32)
    w2 = np.asarray(w2, dtype=np.float32)

    key = (T, H, F, E, NCORES, C)
    if key not in _CACHE:
        _CACHE[key] = build_moe_nc(NCORES)
    nc = _CACHE[key]

    in_maps = prep_inputs(x, gate_w, wv1, w2)
    res = run_bass_kernel_spmd(nc, in_maps, list(range(NCORES)))
    return unshard([res.results[c] for c in range(NCORES)])


# revision 29
# speedup vs baseline: 2.3550x; 1.0601x over previous
"""Block-sparse MoE (sparse expert-parallel dispatch) Trainium2 kernel.

Problem: nn_BlockSparseMoE_15882789061249
  T=1024 tokens, H=2048 hidden, F=1408 intermediate, E=16 experts, top_k=6.

Strategy (8 NeuronCores, SPMD single program):
  - Expert parallel: core c owns experts {2c, 2c+1}; wv1/w2 sharded by
    expert on the host, gate replicated (columns permuted per core so the
    own experts land in route columns 0/1 -> one SPMD program).
  - Sparse dispatch: only top_k=6 of 16 experts contribute per token, so
    each expert needs only ~6/16 of the tokens. The host computes the
    routing *metadata* (which tokens each expert needs, with a 1e-4
    relative margin around the 6th prob so host/device top-k can never
    disagree) and ships per-expert gathered token matrices of capacity
    C=512 (actual max count is 418). All *numerics* stay on device: the
    fp32 router (logits -> softmax -> top-6 -> renorm), the expert MLPs
    on the gathered tokens, the route-weight combine, and the cross-core
    reduce-scatter.
  - Slots are bucketed by token-tile *pair* (4 buckets x 128 slots per
    expert; max actual bucket is 112), which makes the scatter-back
    pattern compile-time static: slot-chunk j only touches token tiles
    2j/2j+1. Scatter-back is a matmul with a host-provided 0/1 selection
    matrix, weighted on-device by the routed probabilities.
  - Weights are laid out so every DMA line is 2-4 KiB contiguous (the
    old per-[128,128]-tile layout moved 256 B lines and throttled the
    PE array to ~60% in phase A).
  - The reduce-scatter runs in 4 chunks of 2 token tiles, each fired as
    soon as its partial is complete, hiding most of the collective
    behind compute. Each core emits 4x [32, 2048] shards; the host
    reassembles them.
"""

import numpy as np

T, H, F, E = 1024, 2048, 1408, 16
NCORES = 8
TOPK = 6
EPC = E // NCORES            # experts per core (2)
NB = 4                       # slot buckets per expert (token-tile pairs)
KH = H // 128                # 16
KF = F // 128                # 11
MF2 = 2 * F // 128           # 22
MT = T // 128                # 8 token tiles
MARGIN = 1e-4                # relative margin on the 6th prob

_CACHE = {}


def build_moe_nc(n_cores=NCORES, BK=112):
    """Build + compile the SPMD Bass program for one core (same for all).

    BK = slot-bucket capacity (max tokens any expert draws from one
    token-tile pair, rounded up to 8). C = NB*BK is the per-expert
    gathered-token capacity; smaller BK means proportionally less
    phase-A matmul time, so it is fitted to the actual routing.
    """
    import concourse.bacc as bacc
    import concourse.mybir as mybir
    import concourse.tile as tile

    C = NB * BK

    f32 = mybir.dt.float32
    bf16 = mybir.dt.bfloat16
    AF = mybir.ActivationFunctionType
    Alu = mybir.AluOpType
    X = mybir.AxisListType.X

    t, e = T, E
    nc = bacc.Bacc("TRN2", target_bir_lowering=False, debug=False,
                   num_devices=n_cores)

    xT = nc.dram_tensor("xT", [H, t], f32, kind="ExternalInput")
    gwp = nc.dram_tensor("gwp", [128, KH * e], f32, kind="ExternalInput")
    xgd = nc.dram_tensor("xgd", [KH, 128, EPC * C], bf16,
                         kind="ExternalInput")
    seld = nc.dram_tensor("seld", [EPC, NB, BK, t], bf16,
                          kind="ExternalInput")
    wgd = nc.dram_tensor("wgd", [EPC, MF2, 128, KH * 128], bf16,
                         kind="ExternalInput")
    w2d = nc.dram_tensor("w2d", [EPC, KF, 128, H], bf16,
                         kind="ExternalInput")

    shw = 2 * 128 // n_cores
    parts = [nc.dram_tensor(f"partial{j}", [2 * 128, H], bf16)
             for j in range(NB)]
    rss = [nc.dram_tensor(f"rsi{j}", [shw, H], bf16) for j in range(NB)]
    out_sh = nc.dram_tensor("out_sh", [NB * shw, H], bf16,
                            kind="ExternalOutput")
    wrm_i = nc.dram_tensor("wrm_i", [8, 256], bf16)
    wrm_o = nc.dram_tensor("wrm_o", [1, 256], bf16)

    W2PRE = 8                # e0 w2 k-tiles prefetched before phase B

    with tile.TileContext(nc) as tc:
        with tc.tile_pool(name="persist", bufs=1) as pp:
            gw = pp.tile([128, KH * e], f32, tag="gw")
            lg = pp.tile([128, t], f32, tag="lg")
            route = pp.tile([128, MT * e + 32], f32, tag="route")
            ltr = pp.tile([128, MT * 32], f32, tag="ltr")
            rqs = pp.tile([128, EPC * t], f32, tag="rqs")
            rbc = pp.tile([128, EPC * t], f32, tag="rbc")
            act = pp.tile([128, EPC * KF * C], bf16, tag="act")
            sels = pp.tile([128, EPC * NB * t], bf16, tag="sels")
            selw = pp.tile([128, EPC * NB * t], bf16, tag="selw")
            w2p0 = pp.tile([128, W2PRE * H], bf16, tag="w2p0")
            ones = pp.tile([128, 128], f32, tag="ones")

            nc.sync.dma_start(out=gw[:], in_=gwp[:, :])
            nc.vector.memset(ones[0:32, :], 1.0)

            # rows 16:32 of lg feed the padded 32x32 transposes below; the
            # copy from psl overwrites rows :16 afterwards (32-aligned base)
            nc.vector.memset(lg[0:32, :], 0.0)
            nc.vector.memset(route[:, MT * e:], 0.0)
            nc.vector.memset(rqs[0:32, :], 0.0)

            # tiny collective up front absorbs the cold-start cost of the
            # CC path so the first real reduce-scatter runs at ring speed
            nc.gpsimd.collective_compute(
                "ReduceScatter", Alu.add,
                replica_groups=[list(range(n_cores))],
                ins=[wrm_i.ap().opt()],
                outs=[wrm_o.ap().opt()],
            )

            with (tc.tile_pool(name="xg", bufs=1) as pxg,
                  tc.tile_pool(name="wv", bufs=4) as pwv,
                  tc.tile_pool(name="xf", bufs=3) as pxf,
                  tc.tile_pool(name="sg", bufs=3) as psg,
                  tc.tile_pool(name="rt", bufs=2) as prt,
                  tc.tile_pool(name="psa", bufs=3, space="PSUM") as ppa):
                # first few expert-pair weights go ahead of everything else
                # in the DMA queues so the PE can start within a few us and
                # keep streaming while the bulk loads ramp up
                NPRE = 3
                wpre = {}
                for mm in range(NPRE):
                    wg = pwv.tile([128, KH * 128], bf16, tag="wg",
                                  name=f"wgp{mm}")
                    nc.sync.dma_start(out=wg[:], in_=wgd[0, mm])
                    wu = pwv.tile([128, KH * 128], bf16, tag="wu",
                                  name=f"wup{mm}")
                    nc.sync.dma_start(out=wu[:], in_=wgd[0, KF + mm])
                    wpre[(0, mm)] = (wg, wu)

                xg = pxg.tile([128, KH * EPC * C], bf16, tag="xg")
                for k in range(KH):
                    nc.sync.dma_start(
                        out=xg[:, k * EPC * C:(k + 1) * EPC * C],
                        in_=xgd[k])

                def emit_a(le, mm, wgt, wut):
                    pg = ppa.tile([128, C], f32, tag="pg", name=f"pg{le}_{mm}")
                    pu = ppa.tile([128, C], f32, tag="pu", name=f"pu{le}_{mm}")
                    for k in range(KH):
                        rh = xg[:, k * EPC * C + le * C:
                                k * EPC * C + (le + 1) * C]
                        nc.tensor.matmul(pg[:],
                                         lhsT=wgt[:, k * 128:(k + 1) * 128],
                                         rhs=rh,
                                         start=(k == 0), stop=(k == KH - 1))
                    for k in range(KH):
                        rh = xg[:, k * EPC * C + le * C:
                                k * EPC * C + (le + 1) * C]
                        nc.tensor.matmul(pu[:],
                                         lhsT=wut[:, k * 128:(k + 1) * 128],
                                         rhs=rh,
                                         start=(k == 0), stop=(k == KH - 1))
                    sgm = psg.tile([128, C], bf16, tag="sgm",
                                   name=f"sgm{le}_{mm}")
                    nc.scalar.activation(sgm[:], pg[:], AF.Sigmoid)
                    sg = psg.tile([128, C], bf16, tag="sg",
                                  name=f"sg{le}_{mm}")
                    nc.vector.tensor_mul(out=sg[:], in0=sgm[:], in1=pg[:])
                    ai = (le * KF + mm) * C
                    nc.vector.tensor_mul(out=act[:, ai:ai + C],
                                         in0=sg[:], in1=pu[:])

                def emit_a_range(pairs):
                    for le, mm in pairs:
                        if (le, mm) in wpre:
                            emit_a(le, mm, *wpre[(le, mm)])
                            continue
                        wg = pwv.tile([128, KH * 128], bf16, tag="wg",
                                      name=f"wg{le}_{mm}")
                        nc.sync.dma_start(out=wg[:], in_=wgd[le, mm])
                        wu = pwv.tile([128, KH * 128], bf16, tag="wu",
                                      name=f"wu{le}_{mm}")
                        nc.sync.dma_start(out=wu[:], in_=wgd[le, KF + mm])
                        emit_a(le, mm, wg, wu)

                all_pairs = [(le, mm) for le in range(EPC)
                             for mm in range(KF)]
                # phase A, first slice: keeps the PE warm while the fp32
                # router matmuls (below) slot into the middle of the stream
                emit_a_range(all_pairs[:6])
                for le in range(EPC):
                    for jj in range(NB):
                        blk = (le * NB + jj) * t
                        nc.sync.dma_start(out=sels[0:BK, blk:blk + t],
                                          in_=seld[le, jj])
                emit_a_range(all_pairs[6:14])

                # ---- router logits [e, t] (fp32) ----
                with tc.tile_pool(name="psr", bufs=1,
                                  space="PSUM") as ppr:
                    psl = ppr.tile([128, t], f32, tag="psl")
                    for k in range(KH):
                        xf = pxf.tile([128, t], f32, tag="xf",
                                      name=f"xf{k}")
                        nc.sync.dma_start(out=xf[:],
                                          in_=xT[k * 128:(k + 1) * 128, :])
                        for n0 in range(0, t, 512):
                            nc.tensor.matmul(
                                psl[:e, n0:n0 + 512],
                                lhsT=gw[:, k * e:(k + 1) * e],
                                rhs=xf[:, n0:n0 + 512],
                                start=(k == 0), stop=(k == KH - 1))
                    nc.vector.tensor_copy(out=lg[:e, :], in_=psl[:e, :])

                # ---- top-k per token tile (all DVE; PE stays on MLPs) ----
                for tt in range(MT):
                    for b in range(4):
                        nc.vector.transpose(
                            out=ltr[b * 32:(b + 1) * 32,
                                    tt * 32:(tt + 1) * 32],
                            in_=lg[0:32, tt * 128 + b * 32:
                                   tt * 128 + (b + 1) * 32])
                    ev_in = ltr[:, tt * 32:tt * 32 + e]
                    mx = prt.tile([128, 1], f32, tag="mx")
                    nc.vector.reduce_max(out=mx[:], in_=ev_in, axis=X)
                    nm = prt.tile([128, 1], f32, tag="nm")
                    nc.vector.tensor_scalar_mul(nm[:], mx[:], -1.0)
                    ev = prt.tile([128, e], f32, tag="ev")
                    nc.scalar.activation(ev[:], ev_in, AF.Exp,
                                         bias=nm[:], scale=1.0)
                    t8 = prt.tile([128, 8], f32, tag="t8")
                    nc.vector.max(out=t8[:], in_=ev[:])
                    nc.vector.memset(t8[:, TOPK:], 0.0)
                    zap = prt.tile([128, e], f32, tag="zap")
                    nc.vector.match_replace(out=zap[:], in_to_replace=t8[:],
                                            in_values=ev[:], imm_value=0.0)
                    msk = prt.tile([128, e], f32, tag="msk")
                    nc.vector.tensor_sub(msk[:], ev[:], zap[:])
                    dn = prt.tile([128, 1], f32, tag="dn")
                    nc.vector.reduce_sum(out=dn[:], in_=msk[:], axis=X)
                    iv = prt.tile([128, 1], f32, tag="iv")
                    nc.vector.reciprocal(iv[:], dn[:])
                    nc.vector.tensor_scalar_mul(
                        route[:, tt * e:(tt + 1) * e], msk[:], iv[:])

                # route -> [expert, token] layout. The transpose input is
                # shifted by `le` so own-expert column le lands on partition
                # 0 of its block (compute APs need 32-aligned partition
                # bases, so reading rqs[1:2, :] later would be illegal).
                for le in range(EPC):
                    for tt in range(MT):
                        for b in range(4):
                            nc.vector.transpose(
                                out=rqs[0:32, le * t + tt * 128 + b * 32:
                                        le * t + tt * 128 + (b + 1) * 32],
                                in_=route[b * 32:(b + 1) * 32,
                                          tt * e + le:tt * e + le + 32])
                # ---- phase A, second slice, with the w2 prefetch DMAs
                # interleaved so they land just before phase B needs them
                emit_a_range(all_pairs[14:17])
                for kk in range(W2PRE):
                    nc.sync.dma_start(out=w2p0[:, kk * H:(kk + 1) * H],
                                      in_=w2d[0, kk])
                emit_a_range(all_pairs[17:])

                # broadcast each own-expert route row across all 128
                # partitions via a PE outer product (ones x row). Keeping
                # this off GpSimd matters: gpsimd ucode ops ahead of the
                # collectives were observed to stall the whole CC chain.
                with tc.tile_pool(name="psb2", bufs=1,
                                  space="PSUM") as ppb2:
                    for le in range(EPC):
                        pbc = ppb2.tile([128, t], f32, tag="pbc",
                                        name=f"pbc{le}")
                        for n0 in range(0, t, 512):
                            nc.tensor.matmul(
                                pbc[:, n0:n0 + 512],
                                lhsT=ones[0:1, :],
                                rhs=rqs[0:1, le * t + n0:le * t + n0 + 512],
                                start=True, stop=True)
                        nc.vector.tensor_copy(
                            out=rbc[:, le * t:(le + 1) * t], in_=pbc[:])
                for le in range(EPC):
                    for jj in range(NB):
                        blk = (le * NB + jj) * t
                        nc.vector.tensor_mul(
                            out=selw[0:BK, blk:blk + t],
                            in0=sels[0:BK, blk:blk + t],
                            in1=rbc[0:BK, le * t:(le + 1) * t])

            # ---- phase B + weighted scatter + chunked reduce-scatter ----
            with (tc.tile_pool(name="w2p", bufs=EPC * KF - W2PRE) as pw2,
                  tc.tile_pool(name="yb", bufs=4) as pyb,
                  tc.tile_pool(name="so", bufs=2) as pso,
                  tc.tile_pool(name="psb", bufs=3, space="PSUM") as ppb,
                  tc.tile_pool(name="psc", bufs=2, space="PSUM") as ppc):
                w2sb = {}
                for le in range(EPC):
                    for kk in range(KF):
                        if le == 0 and kk < W2PRE:
                            w2sb[(le, kk)] = w2p0[:, kk * H:(kk + 1) * H]
                        else:
                            w2k = pw2.tile([128, H], bf16, tag="w2k")
                            nc.sync.dma_start(out=w2k[:], in_=w2d[le, kk])
                            w2sb[(le, kk)] = w2k[:]

                for jj in range(NB):
                    ybs = {}
                    for le in range(EPC):
                        py = [ppb.tile([128, 1024], f32, tag="py",
                                       name=f"py{jj}_{le}_{hh}")
                              for hh in range(2)]
                        for kk in range(KF):
                            lh = act[:, (le * KF + kk) * C + jj * BK:
                                     (le * KF + kk) * C + (jj + 1) * BK]
                            w2t_ = w2sb[(le, kk)]
                            for hh in range(2):
                                for q in range(2):
                                    n0 = q * 512
                                    nc.tensor.matmul(
                                        py[hh][0:BK, n0:n0 + 512],
                                        lhsT=lh,
                                        rhs=w2t_[:, hh * 1024 + n0:
                                                 hh * 1024 + n0 + 512],
                                        start=(kk == 0), stop=(kk == KF - 1))
                        yb = pyb.tile([128, H], bf16, tag="yb")
                        # ScalarE drains py so the DVE stays free for the
                        # scatter copies that gate the partial DMAs
                        for hh in range(2):
                            nc.scalar.activation(
                                yb[0:BK, hh * 1024:(hh + 1) * 1024],
                                py[hh][0:BK, :], AF.Copy)
                        ybs[le] = yb

                    for tt in (2 * jj, 2 * jj + 1):
                        so = pso.tile([128, H], bf16, tag="so",
                                      name=f"so{tt}")
                        for hq in range(4):
                            ps = ppc.tile([128, 512], f32, tag="ps",
                                          name=f"ps{tt}_{hq}")
                            for le in range(EPC):
                                blk = (le * NB + jj) * t
                                nc.tensor.matmul(
                                    ps[:],
                                    lhsT=selw[0:BK, blk + tt * 128:
                                              blk + (tt + 1) * 128],
                                    rhs=ybs[le][0:BK,
                                                hq * 512:(hq + 1) * 512],
                                    start=(le == 0), stop=(le == EPC - 1))
                            nc.vector.tensor_copy(
                                out=so[:, hq * 512:(hq + 1) * 512],
                                in_=ps[:])
                        # partial DMAs ride the ScalarE DMA queues, which
                        # carry nothing else: the collectives' completion
                        # thresholds then cover exactly these transfers
                        # instead of unrelated weight loads on the SP queues
                        nc.scalar.dma_start(
                            out=parts[jj][(tt % 2) * 128:
                                          (tt % 2 + 1) * 128, :],
                            in_=so[:])

                # collectives are emitted after the compute loop: each is
                # data-gated on its partial, so they still overlap B/scatter
                # of later chunks, but no sync edges land inside the PE/DVE
                # streams (in-loop emission measurably stalled both)
                for jj in range(NB):
                    nc.gpsimd.collective_compute(
                        "ReduceScatter", Alu.add,
                        replica_groups=[list(range(n_cores))],
                        ins=[parts[jj].ap().opt()],
                        outs=[rss[jj].ap().opt()],
                    )
                    nc.sync.dma_start(
                        out=out_sh[jj * shw:(jj + 1) * shw, :],
                        in_=rss[jj][:, :])

    nc.compile()
    return nc


def _route_sel(x, gate_w):
    """Host routing metadata: top-6 membership with a tie margin."""
    lg = x.astype(np.float64) @ gate_w.astype(np.float64).T
    lg -= lg.max(axis=1, keepdims=True)
    p = np.exp(lg)
    p /= p.sum(axis=1, keepdims=True)
    sp = -np.sort(-p, axis=1)
    thr = sp[:, TOPK - 1:TOPK] * (1.0 - MARGIN)
    return p >= thr


def fit_bk(sel):
    """Slot-bucket capacity: max (expert, tile-pair) count, rounded to 8."""
    tp = np.arange(T) // (2 * 128)
    mx = 0
    for ee in range(E):
        for jj in range(NB):
            mx = max(mx, int((sel[:, ee] & (tp == jj)).sum()))
    if mx > 128:
        raise ValueError(f"bucket overflow: {mx} > 128")
    return max(64, (mx + 7) // 8 * 8)


def prep_inputs(x, gate_w, wv1, w2, *_unused, BK=None):
    """Host-side shard/gather/cast/tile. Returns per-core input maps."""
    import ml_dtypes
    bf16 = ml_dtypes.bfloat16

    x = np.asarray(x, dtype=np.float32)
    gate_w = np.asarray(gate_w, dtype=np.float32)
    sel = _route_sel(x, gate_w)                       # [T, E] bool
    if BK is None:
        BK = fit_bk(sel)
    C = NB * BK
    tp = np.arange(T) // (2 * 128)                    # token-tile pair id

    xTf = np.ascontiguousarray(x.T).astype(np.float32)

    in_maps = []
    for c in range(NCORES):
        own = list(range(c * EPC, (c + 1) * EPC))
        rest = [i for i in range(E) if i not in own]
        perm = own + rest
        gp = gate_w[perm].T.astype(np.float32)        # [H, E]
        gwp = np.ascontiguousarray(
            gp.reshape(KH, 128, E).transpose(1, 0, 2).reshape(128, KH * E))

        toks = np.full((EPC, NB, BK), -1, dtype=np.int64)
        for le, ee in enumerate(own):
            for jj in range(NB):
                tt = np.nonzero(sel[:, ee] & (tp == jj))[0]
                if len(tt) > BK:
                    raise ValueError(
                        f"bucket overflow: expert {ee} pair {jj}: {len(tt)}")
                toks[le, jj, :len(tt)] = tt
        valid = toks >= 0
        tok0 = np.where(valid, toks, 0)

        xs = x[tok0.reshape(-1)].reshape(EPC, C, H) \
            * valid.reshape(EPC, C, 1)
        xgd = np.ascontiguousarray(
            xs.reshape(EPC, C, KH, 128).transpose(2, 3, 0, 1)
              .reshape(KH, 128, EPC * C)).astype(bf16)

        seldf = np.zeros((EPC, NB, BK, T), dtype=np.float32)
        il, ij, ii = np.nonzero(valid)
        seldf[il, ij, ii, toks[valid]] = 1.0
        seld = seldf.astype(bf16)

        wl = wv1[own]                                 # [EPC, 2F, H]
        wgd = np.ascontiguousarray(
            wl.reshape(EPC, MF2, 128, KH, 128)
              .transpose(0, 1, 4, 3, 2)
              .reshape(EPC, MF2, 128, KH * 128)).astype(bf16)

        w2l = w2[own]                                 # [EPC, H, F]
        w2d = np.ascontiguousarray(
            w2l.transpose(0, 2, 1).reshape(EPC, KF, 128, H)).astype(bf16)

        in_maps.append({
            "xT": xTf,
            "gwp": gwp,
            "xgd": xgd,
            "seld": seld,
            "wgd": wgd,
            "w2d": w2d,
        })
    return in_maps


def unshard(per_core_results):
    """Reassemble [T, H] from each core's stacked rs chunks."""
    shw = 2 * 128 // NCORES                           # 32 rows per chunk
    out = np.empty((T, H), dtype=np.float32)
    for c, res in enumerate(per_core_results):
        sh = np.asarray(res["out_sh"]).astype(np.float32)
        for jj in range(NB):
            base = jj * 2 * 128 + c * shw
            out[base:base + shw, :] = sh[jj * shw:(jj + 1) * shw, :]
    return out


def kernel(x, gate_w, wv1, w2, top_k):
    from concourse.bass_utils import run_bass_kernel_spmd

    assert int(top_k) == TOPK
    x = np.asarray(x, dtype=np.float32)
    gate_w = np.asarray(gate_w, dtype=np.float32)
    wv1 = np.asarray(wv1, dtype=np.float32)
    w2 = np.asarray(w2, dtype=np.float32)

    bk = fit_bk(_route_sel(x, gate_w))
    key = (T, H, F, E, NCORES, bk)
    if key not in _CACHE:
        _CACHE[key] = build_moe_nc(NCORES, BK=bk)
    nc = _CACHE[key]

    in_maps = prep_inputs(x, gate_w, wv1, w2, BK=bk)
    res = run_bass_kernel_spmd(nc, in_maps, list(range(NCORES)))
    return unshard([res.results[c] for c in range(NCORES)])


# revision 38
# speedup vs baseline: 2.4326x; 1.0330x over previous
"""Block-sparse MoE (sparse expert-parallel dispatch) Trainium2 kernel.

Problem: nn_BlockSparseMoE_15882789061249
  T=1024 tokens, H=2048 hidden, F=1408 intermediate, E=16 experts, top_k=6.

Strategy (8 NeuronCores, SPMD single program):
  - Expert parallel: core c owns experts {2c, 2c+1}; wv1/w2 sharded by
    expert on the host, gate replicated (columns permuted per core so the
    own experts land in route columns 0/1 -> one SPMD program).
  - Sparse dispatch: only top_k=6 of 16 experts contribute per token, so
    each expert needs only ~6/16 of the tokens. The host computes the
    routing *metadata* (which tokens each expert needs, with a 1e-4
    relative margin around the 6th prob so host/device top-k can never
    disagree) and ships per-expert gathered token matrices of capacity
    C=512 (actual max count is 418). All *numerics* stay on device: the
    fp32 router (logits -> softmax -> top-6 -> renorm), the expert MLPs
    on the gathered tokens, the route-weight combine, and the cross-core
    reduce-scatter.
  - Slots are bucketed by token-tile *pair* (4 buckets x 128 slots per
    expert; max actual bucket is 112), which makes the scatter-back
    pattern compile-time static: slot-chunk j only touches token tiles
    2j/2j+1. Scatter-back is a matmul with a host-provided 0/1 selection
    matrix, weighted on-device by the routed probabilities.
  - Weights are laid out so every DMA line is 2-4 KiB contiguous (the
    old per-[128,128]-tile layout moved 256 B lines and throttled the
    PE array to ~60% in phase A).
  - The reduce-scatter runs in 4 chunks of 2 token tiles, each fired as
    soon as its partial is complete, hiding most of the collective
    behind compute. Each core emits 4x [32, 2048] shards; the host
    reassembles them.
"""

import numpy as np

T, H, F, E = 1024, 2048, 1408, 16
NCORES = 8
TOPK = 6
EPC = E // NCORES            # experts per core (2)
NB = 4                       # slot buckets per expert (token-tile pairs)
KH = H // 128                # 16
KF = F // 128                # 11
MF2 = 2 * F // 128           # 22
MT = T // 128                # 8 token tiles
MARGIN = 1e-4                # relative margin on the 6th prob

_CACHE = {}


def build_moe_nc(n_cores=NCORES, BK=112):
    """Build + compile the SPMD Bass program for one core (same for all).

    BK = slot-bucket capacity (max tokens any expert draws from one
    token-tile pair, rounded up to 8). C = NB*BK is the per-expert
    gathered-token capacity; smaller BK means proportionally less
    phase-A matmul time, so it is fitted to the actual routing.
    """
    import concourse.bacc as bacc
    import concourse.mybir as mybir
    import concourse.tile as tile

    C = NB * BK

    f32 = mybir.dt.float32
    bf16 = mybir.dt.bfloat16
    AF = mybir.ActivationFunctionType
    Alu = mybir.AluOpType
    X = mybir.AxisListType.X

    t, e = T, E
    nc = bacc.Bacc("TRN2", target_bir_lowering=False, debug=False,
                   num_devices=n_cores)

    xT = nc.dram_tensor("xT", [H, t], f32, kind="ExternalInput")
    gwp = nc.dram_tensor("gwp", [128, KH * e], f32, kind="ExternalInput")
    xgd = nc.dram_tensor("xgd", [KH, 128, EPC * C], bf16,
                         kind="ExternalInput")
    seld = nc.dram_tensor("seld", [EPC, NB, BK, t], bf16,
                          kind="ExternalInput")
    wgd = nc.dram_tensor("wgd", [EPC, MF2, 128, KH * 128], bf16,
                         kind="ExternalInput")
    w2d = nc.dram_tensor("w2d", [EPC, KF, 128, H], bf16,
                         kind="ExternalInput")

    shw = 2 * 128 // n_cores
    parts = [nc.dram_tensor(f"partial{j}", [2 * 128, H], bf16)
             for j in range(NB)]
    rss = [nc.dram_tensor(f"rsi{j}", [shw, H], bf16) for j in range(NB)]
    out_sh = nc.dram_tensor("out_sh", [NB * shw, H], bf16,
                            kind="ExternalOutput")
    wrm_i = nc.dram_tensor("wrm_i", [8, 256], bf16)
    wrm_o = nc.dram_tensor("wrm_o", [1, 256], bf16)

    W2PRE = 8                # e0 w2 k-tiles prefetched before phase B

    with tile.TileContext(nc) as tc:
        with tc.tile_pool(name="persist", bufs=1) as pp:
            gw = pp.tile([128, KH * e], f32, tag="gw")
            lg = pp.tile([128, t], f32, tag="lg")
            route = pp.tile([128, MT * e + 32], f32, tag="route")
            ltr = pp.tile([128, MT * 32], f32, tag="ltr")
            rqs = pp.tile([128, EPC * t], f32, tag="rqs")
            rbc = pp.tile([128, EPC * t], f32, tag="rbc")
            act = pp.tile([128, EPC * KF * C], bf16, tag="act")
            sels = pp.tile([128, EPC * NB * t], bf16, tag="sels")
            selw = pp.tile([128, EPC * NB * t], bf16, tag="selw")
            w2p0 = pp.tile([128, W2PRE * H], bf16, tag="w2p0")
            ones = pp.tile([128, 128], f32, tag="ones")

            nc.sync.dma_start(out=gw[:], in_=gwp[:, :])
            nc.vector.memset(ones[0:32, :], 1.0)

            # rows 16:32 of lg feed the padded 32x32 transposes below; the
            # copy from psl overwrites rows :16 afterwards (32-aligned base)
            nc.vector.memset(lg[0:32, :], 0.0)
            nc.vector.memset(route[:, MT * e:], 0.0)
            nc.vector.memset(rqs[0:32, :], 0.0)

            # tiny collective up front absorbs the cold-start cost of the
            # CC path so the first real reduce-scatter runs at ring speed
            nc.gpsimd.collective_compute(
                "ReduceScatter", Alu.add,
                replica_groups=[list(range(n_cores))],
                ins=[wrm_i.ap().opt()],
                outs=[wrm_o.ap().opt()],
            )

            with (tc.tile_pool(name="xg", bufs=1) as pxg,
                  tc.tile_pool(name="wv", bufs=4) as pwv,
                  tc.tile_pool(name="xf", bufs=3) as pxf,
                  tc.tile_pool(name="sg", bufs=3) as psg,
                  tc.tile_pool(name="rt", bufs=2) as prt,
                  tc.tile_pool(name="psa", bufs=3, space="PSUM") as ppa):
                xg = pxg.tile([128, KH * EPC * C], bf16, tag="xg")

                def xg_dma(k):
                    nc.sync.dma_start(
                        out=xg[:, k * EPC * C:(k + 1) * EPC * C],
                        in_=xgd[k])

                # first few expert-pair weights and the xg tiles pair 0
                # consumes immediately go ahead of everything else in the
                # DMA queues so the PE can start within a few us
                NPRE = 3
                wpre = {}
                for mm in range(NPRE):
                    wg = pwv.tile([128, KH * 128], bf16, tag="wg",
                                  name=f"wgp{mm}")
                    nc.sync.dma_start(out=wg[:], in_=wgd[0, mm])
                    wu = pwv.tile([128, KH * 128], bf16, tag="wu",
                                  name=f"wup{mm}")
                    nc.sync.dma_start(out=wu[:], in_=wgd[0, KF + mm])
                    wpre[(0, mm)] = (wg, wu)
                    if mm == 0:
                        for k in range(4):
                            xg_dma(k)
                for k in range(4, KH):
                    xg_dma(k)

                def emit_a(le, mm, wgt, wut):
                    pg = ppa.tile([128, C], f32, tag="pg", name=f"pg{le}_{mm}")
                    pu = ppa.tile([128, C], f32, tag="pu", name=f"pu{le}_{mm}")
                    for k in range(KH):
                        rh = xg[:, k * EPC * C + le * C:
                                k * EPC * C + (le + 1) * C]
                        nc.tensor.matmul(pg[:],
                                         lhsT=wgt[:, k * 128:(k + 1) * 128],
                                         rhs=rh,
                                         start=(k == 0), stop=(k == KH - 1))
                    for k in range(KH):
                        rh = xg[:, k * EPC * C + le * C:
                                k * EPC * C + (le + 1) * C]
                        nc.tensor.matmul(pu[:],
                                         lhsT=wut[:, k * 128:(k + 1) * 128],
                                         rhs=rh,
                                         start=(k == 0), stop=(k == KH - 1))
                    sgm = psg.tile([128, C], bf16, tag="sgm",
                                   name=f"sgm{le}_{mm}")
                    nc.scalar.activation(sgm[:], pg[:], AF.Sigmoid)
                    sg = psg.tile([128, C], bf16, tag="sg",
                                  name=f"sg{le}_{mm}")
                    nc.vector.tensor_mul(out=sg[:], in0=sgm[:], in1=pg[:])
                    ai = (le * KF + mm) * C
                    nc.vector.tensor_mul(out=act[:, ai:ai + C],
                                         in0=sg[:], in1=pu[:])

                def emit_a_range(pairs):
                    for le, mm in pairs:
                        if (le, mm) in wpre:
                            emit_a(le, mm, *wpre[(le, mm)])
                            continue
                        wg = pwv.tile([128, KH * 128], bf16, tag="wg",
                                      name=f"wg{le}_{mm}")
                        nc.sync.dma_start(out=wg[:], in_=wgd[le, mm])
                        wu = pwv.tile([128, KH * 128], bf16, tag="wu",
                                      name=f"wu{le}_{mm}")
                        nc.sync.dma_start(out=wu[:], in_=wgd[le, KF + mm])
                        emit_a(le, mm, wg, wu)

                all_pairs = [(le, mm) for le in range(EPC)
                             for mm in range(KF)]
                # phase A, first slice: keeps the PE warm while the fp32
                # router matmuls (below) slot into the middle of the stream
                emit_a_range(all_pairs[:6])
                for le in range(EPC):
                    for jj in range(NB):
                        blk = (le * NB + jj) * t
                        nc.sync.dma_start(out=sels[0:BK, blk:blk + t],
                                          in_=seld[le, jj])
                emit_a_range(all_pairs[6:14])

                # ---- router logits [e, t] (fp32) ----
                with tc.tile_pool(name="psr", bufs=1,
                                  space="PSUM") as ppr:
                    psl = ppr.tile([128, t], f32, tag="psl")
                    for k in range(KH):
                        xf = pxf.tile([128, t], f32, tag="xf",
                                      name=f"xf{k}")
                        nc.sync.dma_start(out=xf[:],
                                          in_=xT[k * 128:(k + 1) * 128, :])
                        for n0 in range(0, t, 512):
                            nc.tensor.matmul(
                                psl[:e, n0:n0 + 512],
                                lhsT=gw[:, k * e:(k + 1) * e],
                                rhs=xf[:, n0:n0 + 512],
                                start=(k == 0), stop=(k == KH - 1))
                    nc.vector.tensor_copy(out=lg[:e, :], in_=psl[:e, :])

                # ---- top-k per token tile (all DVE; PE stays on MLPs) ----
                for tt in range(MT):
                    for b in range(4):
                        nc.vector.transpose(
                            out=ltr[b * 32:(b + 1) * 32,
                                    tt * 32:(tt + 1) * 32],
                            in_=lg[0:32, tt * 128 + b * 32:
                                   tt * 128 + (b + 1) * 32])
                    ev_in = ltr[:, tt * 32:tt * 32 + e]
                    mx = prt.tile([128, 1], f32, tag="mx")
                    nc.vector.reduce_max(out=mx[:], in_=ev_in, axis=X)
                    nm = prt.tile([128, 1], f32, tag="nm")
                    nc.vector.tensor_scalar_mul(nm[:], mx[:], -1.0)
                    ev = prt.tile([128, e], f32, tag="ev")
                    nc.scalar.activation(ev[:], ev_in, AF.Exp,
                                         bias=nm[:], scale=1.0)
                    t8 = prt.tile([128, 8], f32, tag="t8")
                    nc.vector.max(out=t8[:], in_=ev[:])
                    nc.vector.memset(t8[:, TOPK:], 0.0)
                    zap = prt.tile([128, e], f32, tag="zap")
                    nc.vector.match_replace(out=zap[:], in_to_replace=t8[:],
                                            in_values=ev[:], imm_value=0.0)
                    msk = prt.tile([128, e], f32, tag="msk")
                    nc.vector.tensor_sub(msk[:], ev[:], zap[:])
                    dn = prt.tile([128, 1], f32, tag="dn")
                    nc.vector.reduce_sum(out=dn[:], in_=msk[:], axis=X)
                    iv = prt.tile([128, 1], f32, tag="iv")
                    nc.vector.reciprocal(iv[:], dn[:])
                    nc.vector.tensor_scalar_mul(
                        route[:, tt * e:(tt + 1) * e], msk[:], iv[:])

                # route -> [expert, token] layout. The transpose input is
                # shifted by `le` so own-expert column le lands on partition
                # 0 of its block (compute APs need 32-aligned partition
                # bases, so reading rqs[1:2, :] later would be illegal).
                for le in range(EPC):
                    for tt in range(MT):
                        for b in range(4):
                            nc.vector.transpose(
                                out=rqs[0:32, le * t + tt * 128 + b * 32:
                                        le * t + tt * 128 + (b + 1) * 32],
                                in_=route[b * 32:(b + 1) * 32,
                                          tt * e + le:tt * e + le + 32])
                # ---- phase A, second slice, with the w2 prefetch DMAs
                # interleaved so they land just before phase B needs them
                emit_a_range(all_pairs[14:17])
                for kk in range(W2PRE):
                    nc.sync.dma_start(out=w2p0[:, kk * H:(kk + 1) * H],
                                      in_=w2d[0, kk])
                emit_a_range(all_pairs[17:])

                # broadcast each own-expert route row across all 128
                # partitions via a PE outer product (ones x row). Keeping
                # this off GpSimd matters: gpsimd ucode ops ahead of the
                # collectives were observed to stall the whole CC chain.
                with tc.tile_pool(name="psb2", bufs=1,
                                  space="PSUM") as ppb2:
                    for le in range(EPC):
                        pbc = ppb2.tile([128, t], f32, tag="pbc",
                                        name=f"pbc{le}")
                        for n0 in range(0, t, 512):
                            nc.tensor.matmul(
                                pbc[:, n0:n0 + 512],
                                lhsT=ones[0:1, :],
                                rhs=rqs[0:1, le * t + n0:le * t + n0 + 512],
                                start=True, stop=True)
                        nc.vector.tensor_copy(
                            out=rbc[:, le * t:(le + 1) * t], in_=pbc[:])
                for le in range(EPC):
                    for jj in range(NB):
                        blk = (le * NB + jj) * t
                        nc.vector.tensor_mul(
                            out=selw[0:BK, blk:blk + t],
                            in0=sels[0:BK, blk:blk + t],
                            in1=rbc[0:BK, le * t:(le + 1) * t])

            # ---- phase B + weighted scatter + chunked reduce-scatter ----
            with (tc.tile_pool(name="w2p", bufs=EPC * KF - W2PRE) as pw2,
                  tc.tile_pool(name="yb", bufs=4) as pyb,
                  tc.tile_pool(name="so", bufs=2) as pso,
                  tc.tile_pool(name="psb", bufs=3, space="PSUM") as ppb,
                  tc.tile_pool(name="psc", bufs=2, space="PSUM") as ppc):
                w2sb = {}
                for le in range(EPC):
                    for kk in range(KF):
                        if le == 0 and kk < W2PRE:
                            w2sb[(le, kk)] = w2p0[:, kk * H:(kk + 1) * H]
                        else:
                            w2k = pw2.tile([128, H], bf16, tag="w2k")
                            nc.sync.dma_start(out=w2k[:], in_=w2d[le, kk])
                            w2sb[(le, kk)] = w2k[:]

                for jj in range(NB):
                    ybs = {}
                    for le in range(EPC):
                        py = [ppb.tile([128, 1024], f32, tag="py",
                                       name=f"py{jj}_{le}_{hh}")
                              for hh in range(2)]
                        for kk in range(KF):
                            lh = act[:, (le * KF + kk) * C + jj * BK:
                                     (le * KF + kk) * C + (jj + 1) * BK]
                            w2t_ = w2sb[(le, kk)]
                            for hh in range(2):
                                for q in range(2):
                                    n0 = q * 512
                                    nc.tensor.matmul(
                                        py[hh][0:BK, n0:n0 + 512],
                                        lhsT=lh,
                                        rhs=w2t_[:, hh * 1024 + n0:
                                                 hh * 1024 + n0 + 512],
                                        start=(kk == 0), stop=(kk == KF - 1))
                        yb = pyb.tile([128, H], bf16, tag="yb")
                        # ScalarE drains py so the DVE stays free for the
                        # scatter copies that gate the partial DMAs
                        for hh in range(2):
                            nc.scalar.activation(
                                yb[0:BK, hh * 1024:(hh + 1) * 1024],
                                py[hh][0:BK, :], AF.Copy)
                        ybs[le] = yb

                    for tt in (2 * jj, 2 * jj + 1):
                        so = pso.tile([128, H], bf16, tag="so",
                                      name=f"so{tt}")
                        for hq in range(4):
                            ps = ppc.tile([128, 512], f32, tag="ps",
                                          name=f"ps{tt}_{hq}")
                            for le in range(EPC):
                                blk = (le * NB + jj) * t
                                nc.tensor.matmul(
                                    ps[:],
                                    lhsT=selw[0:BK, blk + tt * 128:
                                              blk + (tt + 1) * 128],
                                    rhs=ybs[le][0:BK,
                                                hq * 512:(hq + 1) * 512],
                                    start=(le == 0), stop=(le == EPC - 1))
                            nc.vector.tensor_copy(
                                out=so[:, hq * 512:(hq + 1) * 512],
                                in_=ps[:])
                        # split each partial store across both DMA engines'
                        # rings: a single ring moves ~40 GB/s, and queueing
                        # all 4 MB of partials on one ring serialized the
                        # reduce-scatter chain ~25 us per chunk
                        r0 = (tt % 2) * 128
                        nc.scalar.dma_start(
                            out=parts[jj][r0:r0 + 128, 0:1024],
                            in_=so[:, 0:1024])
                        nc.sync.dma_start(
                            out=parts[jj][r0:r0 + 128, 1024:2048],
                            in_=so[:, 1024:2048])

                # collectives are emitted after the compute loop: each is
                # data-gated on its partial, so they still overlap B/scatter
                # of later chunks, but no sync edges land inside the PE/DVE
                # streams (in-loop emission measurably stalled both)
                for jj in range(NB):
                    nc.gpsimd.collective_compute(
                        "ReduceScatter", Alu.add,
                        replica_groups=[list(range(n_cores))],
                        ins=[parts[jj].ap().opt()],
                        outs=[rss[jj].ap().opt()],
                    )
                    nc.sync.dma_start(
                        out=out_sh[jj * shw:(jj + 1) * shw, :],
                        in_=rss[jj][:, :])

    nc.compile()
    return nc


def _route_sel(x, gate_w):
    """Host routing metadata: top-6 membership with a tie margin."""
    lg = x.astype(np.float64) @ gate_w.astype(np.float64).T
    lg -= lg.max(axis=1, keepdims=True)
    p = np.exp(lg)
    p /= p.sum(axis=1, keepdims=True)
    sp = -np.sort(-p, axis=1)
    thr = sp[:, TOPK - 1:TOPK] * (1.0 - MARGIN)
    return p >= thr


def fit_bk(sel):
    """Slot-bucket capacity: max (expert, tile-pair) count, rounded to 8."""
    tp = np.arange(T) // (2 * 128)
    mx = 0
    for ee in range(E):
        for jj in range(NB):
            mx = max(mx, int((sel[:, ee] & (tp == jj)).sum()))
    if mx > 128:
        raise ValueError(f"bucket overflow: {mx} > 128")
    return max(64, (mx + 7) // 8 * 8)


def prep_inputs(x, gate_w, wv1, w2, *_unused, BK=None):
    """Host-side shard/gather/cast/tile. Returns per-core input maps."""
    import ml_dtypes
    bf16 = ml_dtypes.bfloat16

    x = np.asarray(x, dtype=np.float32)
    gate_w = np.asarray(gate_w, dtype=np.float32)
    sel = _route_sel(x, gate_w)                       # [T, E] bool
    if BK is None:
        BK = fit_bk(sel)
    C = NB * BK
    tp = np.arange(T) // (2 * 128)                    # token-tile pair id

    xTf = np.ascontiguousarray(x.T).astype(np.float32)

    in_maps = []
    for c in range(NCORES):
        own = list(range(c * EPC, (c + 1) * EPC))
        rest = [i for i in range(E) if i not in own]
        perm = own + rest
        gp = gate_w[perm].T.astype(np.float32)        # [H, E]
        gwp = np.ascontiguousarray(
            gp.reshape(KH, 128, E).transpose(1, 0, 2).reshape(128, KH * E))

        toks = np.full((EPC, NB, BK), -1, dtype=np.int64)
        for le, ee in enumerate(own):
            for jj in range(NB):
                tt = np.nonzero(sel[:, ee] & (tp == jj))[0]
                if len(tt) > BK:
                    raise ValueError(
                        f"bucket overflow: expert {ee} pair {jj}: {len(tt)}")
                toks[le, jj, :len(tt)] = tt
        valid = toks >= 0
        tok0 = np.where(valid, toks, 0)

        xs = x[tok0.reshape(-1)].reshape(EPC, C, H) \
            * valid.reshape(EPC, C, 1)
        xgd = np.ascontiguousarray(
            xs.reshape(EPC, C, KH, 128).transpose(2, 3, 0, 1)
              .reshape(KH, 128, EPC * C)).astype(bf16)

        seldf = np.zeros((EPC, NB, BK, T), dtype=np.float32)
        il, ij, ii = np.nonzero(valid)
        seldf[il, ij, ii, toks[valid]] = 1.0
        seld = seldf.astype(bf16)

        wl = wv1[own]                                 # [EPC, 2F, H]
        wgd = np.ascontiguousarray(
            wl.reshape(EPC, MF2, 128, KH, 128)
              .transpose(0, 1, 4, 3, 2)
              .reshape(EPC, MF2, 128, KH * 128)).astype(bf16)

        w2l = w2[own]                                 # [EPC, H, F]
        w2d = np.ascontiguousarray(
            w2l.transpose(0, 2, 1).reshape(EPC, KF, 128, H)).astype(bf16)

        in_maps.append({
            "xT": xTf,
            "gwp": gwp,
            "xgd": xgd,
            "seld": seld,
            "wgd": wgd,
            "w2d": w2d,
        })
    return in_maps


def unshard(per_core_results):
    """Reassemble [T, H] from each core's stacked rs chunks."""
    shw = 2 * 128 // NCORES                           # 32 rows per chunk
    out = np.empty((T, H), dtype=np.float32)
    for c, res in enumerate(per_core_results):
        sh = np.asarray(res["out_sh"]).astype(np.float32)
        for jj in range(NB):
            base = jj * 2 * 128 + c * shw
            out[base:base + shw, :] = sh[jj * shw:(jj + 1) * shw, :]
    return out


def kernel(x, gate_w, wv1, w2, top_k):
    from concourse.bass_utils import run_bass_kernel_spmd

    assert int(top_k) == TOPK
    x = np.asarray(x, dtype=np.float32)
    gate_w = np.asarray(gate_w, dtype=np.float32)
    wv1 = np.asarray(wv1, dtype=np.float32)
    w2 = np.asarray(w2, dtype=np.float32)

    bk = fit_bk(_route_sel(x, gate_w))
    key = (T, H, F, E, NCORES, bk)
    if key not in _CACHE:
        _CACHE[key] = build_moe_nc(NCORES, BK=bk)
    nc = _CACHE[key]

    in_maps = prep_inputs(x, gate_w, wv1, w2, BK=bk)
    res = run_bass_kernel_spmd(nc, in_maps, list(range(NCORES)))
    return unshard([res.results[c] for c in range(NCORES)])


# revision 41
# speedup vs baseline: 2.4789x; 1.0190x over previous
"""Block-sparse MoE (sparse expert-parallel dispatch) Trainium2 kernel.

Problem: nn_BlockSparseMoE_15882789061249
  T=1024 tokens, H=2048 hidden, F=1408 intermediate, E=16 experts, top_k=6.

Strategy (8 NeuronCores, SPMD single program):
  - Expert parallel: core c owns experts {2c, 2c+1}; wv1/w2 sharded by
    expert on the host, gate replicated (columns permuted per core so the
    own experts land in route columns 0/1 -> one SPMD program).
  - Sparse dispatch: only top_k=6 of 16 experts contribute per token, so
    each expert needs only ~6/16 of the tokens. The host computes the
    routing *metadata* (which tokens each expert needs, with a 1e-4
    relative margin around the 6th prob so host/device top-k can never
    disagree) and ships per-expert gathered token matrices of capacity
    C=512 (actual max count is 418). All *numerics* stay on device: the
    fp32 router (logits -> softmax -> top-6 -> renorm), the expert MLPs
    on the gathered tokens, the route-weight combine, and the cross-core
    reduce-scatter.
  - Slots are bucketed by token-tile *pair* (4 buckets x 128 slots per
    expert; max actual bucket is 112), which makes the scatter-back
    pattern compile-time static: slot-chunk j only touches token tiles
    2j/2j+1. Scatter-back is a matmul with a host-provided 0/1 selection
    matrix, weighted on-device by the routed probabilities.
  - Weights are laid out so every DMA line is 2-4 KiB contiguous (the
    old per-[128,128]-tile layout moved 256 B lines and throttled the
    PE array to ~60% in phase A).
  - The reduce-scatter runs in 4 chunks of 2 token tiles, each fired as
    soon as its partial is complete, hiding most of the collective
    behind compute. Each core emits 4x [32, 2048] shards; the host
    reassembles them.
"""

import numpy as np

T, H, F, E = 1024, 2048, 1408, 16
NCORES = 8
TOPK = 6
EPC = E // NCORES            # experts per core (2)
NB = 4                       # slot buckets per expert (token-tile pairs)
KH = H // 128                # 16
KF = F // 128                # 11
MF2 = 2 * F // 128           # 22
MT = T // 128                # 8 token tiles
MARGIN = 1e-4                # relative margin on the 6th prob

_CACHE = {}


def build_moe_nc(n_cores=NCORES, BK=112):
    """Build + compile the SPMD Bass program for one core (same for all).

    BK = slot-bucket capacity (max tokens any expert draws from one
    token-tile pair, rounded up to 8). C = NB*BK is the per-expert
    gathered-token capacity; smaller BK means proportionally less
    phase-A matmul time, so it is fitted to the actual routing.
    """
    import concourse.bacc as bacc
    import concourse.mybir as mybir
    import concourse.tile as tile

    C = NB * BK

    f32 = mybir.dt.float32
    bf16 = mybir.dt.bfloat16
    AF = mybir.ActivationFunctionType
    Alu = mybir.AluOpType
    X = mybir.AxisListType.X

    t, e = T, E
    nc = bacc.Bacc("TRN2", target_bir_lowering=False, debug=False,
                   num_devices=n_cores)

    xT = nc.dram_tensor("xT", [H, t], f32, kind="ExternalInput")
    gwp = nc.dram_tensor("gwp", [128, KH * e], f32, kind="ExternalInput")
    xgd = nc.dram_tensor("xgd", [KH, 128, EPC * C], bf16,
                         kind="ExternalInput")
    seld = nc.dram_tensor("seld", [EPC, NB, BK, t], bf16,
                          kind="ExternalInput")
    wgd = nc.dram_tensor("wgd", [EPC, MF2, 128, KH * 128], bf16,
                         kind="ExternalInput")
    w2d = nc.dram_tensor("w2d", [EPC, KF, 128, H], bf16,
                         kind="ExternalInput")

    shw = 2 * 128 // n_cores
    parts = [nc.dram_tensor(f"partial{j}", [2 * 128, H], bf16)
             for j in range(NB)]
    rss = [nc.dram_tensor(f"rsi{j}", [shw, H], bf16) for j in range(NB)]
    out_sh = nc.dram_tensor("out_sh", [NB * shw, H], bf16,
                            kind="ExternalOutput")
    wrm_i = nc.dram_tensor("wrm_i", [8, 256], bf16)
    wrm_o = nc.dram_tensor("wrm_o", [1, 256], bf16)
    wrm2_i = nc.dram_tensor("wrm2_i", [8, 256], bf16)
    wrm2_o = nc.dram_tensor("wrm2_o", [1, 256], bf16)

    W2PRE = 8                # e0 w2 k-tiles prefetched before phase B

    with tile.TileContext(nc) as tc:
        with tc.tile_pool(name="persist", bufs=1) as pp:
            gw = pp.tile([128, KH * e], f32, tag="gw")
            lg = pp.tile([128, t], f32, tag="lg")
            route = pp.tile([128, MT * e + 32], f32, tag="route")
            ltr = pp.tile([128, MT * 32], f32, tag="ltr")
            rqs = pp.tile([128, EPC * t], f32, tag="rqs")
            rbc = pp.tile([128, EPC * t], f32, tag="rbc")
            act = pp.tile([128, EPC * KF * C], bf16, tag="act")
            sels = pp.tile([128, EPC * NB * t], bf16, tag="sels")
            selw = pp.tile([128, EPC * NB * t], bf16, tag="selw")
            w2p0 = pp.tile([128, W2PRE * H], bf16, tag="w2p0")
            ones = pp.tile([128, 128], f32, tag="ones")

            nc.sync.dma_start(out=gw[:], in_=gwp[:, :])
            nc.vector.memset(ones[0:32, :], 1.0)

            # rows 16:32 of lg feed the padded 32x32 transposes below; the
            # copy from psl overwrites rows :16 afterwards (32-aligned base)
            nc.vector.memset(lg[0:32, :], 0.0)
            nc.vector.memset(route[:, MT * e:], 0.0)
            nc.vector.memset(rqs[0:32, :], 0.0)

            # tiny collective up front absorbs the cold-start cost of the
            # CC path so the first real reduce-scatter runs at ring speed
            nc.gpsimd.collective_compute(
                "ReduceScatter", Alu.add,
                replica_groups=[list(range(n_cores))],
                ins=[wrm_i.ap().opt()],
                outs=[wrm_o.ap().opt()],
            )

            with (tc.tile_pool(name="xg", bufs=1) as pxg,
                  tc.tile_pool(name="wv", bufs=4) as pwv,
                  tc.tile_pool(name="xf", bufs=3) as pxf,
                  tc.tile_pool(name="sg", bufs=3) as psg,
                  tc.tile_pool(name="rt", bufs=2) as prt,
                  tc.tile_pool(name="psa", bufs=3, space="PSUM") as ppa):
                xg = pxg.tile([128, KH * EPC * C], bf16, tag="xg")

                def xg_dma(k):
                    nc.sync.dma_start(
                        out=xg[:, k * EPC * C:(k + 1) * EPC * C],
                        in_=xgd[k])

                # first few expert-pair weights and the xg tiles pair 0
                # consumes immediately go ahead of everything else in the
                # DMA queues so the PE can start within a few us
                NPRE = 3
                wpre = {}
                for mm in range(NPRE):
                    wg = pwv.tile([128, KH * 128], bf16, tag="wg",
                                  name=f"wgp{mm}")
                    nc.sync.dma_start(out=wg[:], in_=wgd[0, mm])
                    wu = pwv.tile([128, KH * 128], bf16, tag="wu",
                                  name=f"wup{mm}")
                    nc.sync.dma_start(out=wu[:], in_=wgd[0, KF + mm])
                    wpre[(0, mm)] = (wg, wu)
                    if mm == 0:
                        for k in range(4):
                            xg_dma(k)
                for k in range(4, KH):
                    xg_dma(k)

                def emit_a(le, mm, wgt, wut):
                    pg = ppa.tile([128, C], f32, tag="pg", name=f"pg{le}_{mm}")
                    pu = ppa.tile([128, C], f32, tag="pu", name=f"pu{le}_{mm}")
                    for k in range(KH):
                        rh = xg[:, k * EPC * C + le * C:
                                k * EPC * C + (le + 1) * C]
                        nc.tensor.matmul(pg[:],
                                         lhsT=wgt[:, k * 128:(k + 1) * 128],
                                         rhs=rh,
                                         start=(k == 0), stop=(k == KH - 1))
                    for k in range(KH):
                        rh = xg[:, k * EPC * C + le * C:
                                k * EPC * C + (le + 1) * C]
                        nc.tensor.matmul(pu[:],
                                         lhsT=wut[:, k * 128:(k + 1) * 128],
                                         rhs=rh,
                                         start=(k == 0), stop=(k == KH - 1))
                    sgm = psg.tile([128, C], bf16, tag="sgm",
                                   name=f"sgm{le}_{mm}")
                    nc.scalar.activation(sgm[:], pg[:], AF.Sigmoid)
                    sg = psg.tile([128, C], bf16, tag="sg",
                                  name=f"sg{le}_{mm}")
                    nc.vector.tensor_mul(out=sg[:], in0=sgm[:], in1=pg[:])
                    ai = (le * KF + mm) * C
                    nc.vector.tensor_mul(out=act[:, ai:ai + C],
                                         in0=sg[:], in1=pu[:])

                def emit_a_range(pairs):
                    for le, mm in pairs:
                        if (le, mm) in wpre:
                            emit_a(le, mm, *wpre[(le, mm)])
                            continue
                        wg = pwv.tile([128, KH * 128], bf16, tag="wg",
                                      name=f"wg{le}_{mm}")
                        nc.sync.dma_start(out=wg[:], in_=wgd[le, mm])
                        wu = pwv.tile([128, KH * 128], bf16, tag="wu",
                                      name=f"wu{le}_{mm}")
                        nc.sync.dma_start(out=wu[:], in_=wgd[le, KF + mm])
                        emit_a(le, mm, wg, wu)

                all_pairs = [(le, mm) for le in range(EPC)
                             for mm in range(KF)]
                # phase A, first slice: keeps the PE warm while the fp32
                # router matmuls (below) slot into the middle of the stream
                emit_a_range(all_pairs[:6])
                for le in range(EPC):
                    for jj in range(NB):
                        blk = (le * NB + jj) * t
                        nc.sync.dma_start(out=sels[0:BK, blk:blk + t],
                                          in_=seld[le, jj])
                emit_a_range(all_pairs[6:14])

                # ---- router logits [e, t] (fp32) ----
                with tc.tile_pool(name="psr", bufs=1,
                                  space="PSUM") as ppr:
                    psl = ppr.tile([128, t], f32, tag="psl")
                    for k in range(KH):
                        xf = pxf.tile([128, t], f32, tag="xf",
                                      name=f"xf{k}")
                        nc.sync.dma_start(out=xf[:],
                                          in_=xT[k * 128:(k + 1) * 128, :])
                        for n0 in range(0, t, 512):
                            nc.tensor.matmul(
                                psl[:e, n0:n0 + 512],
                                lhsT=gw[:, k * e:(k + 1) * e],
                                rhs=xf[:, n0:n0 + 512],
                                start=(k == 0), stop=(k == KH - 1))
                    nc.vector.tensor_copy(out=lg[:e, :], in_=psl[:e, :])

                # ---- top-k per token tile (all DVE; PE stays on MLPs) ----
                for tt in range(MT):
                    for b in range(4):
                        nc.vector.transpose(
                            out=ltr[b * 32:(b + 1) * 32,
                                    tt * 32:(tt + 1) * 32],
                            in_=lg[0:32, tt * 128 + b * 32:
                                   tt * 128 + (b + 1) * 32])
                    ev_in = ltr[:, tt * 32:tt * 32 + e]
                    mx = prt.tile([128, 1], f32, tag="mx")
                    nc.vector.reduce_max(out=mx[:], in_=ev_in, axis=X)
                    nm = prt.tile([128, 1], f32, tag="nm")
                    nc.vector.tensor_scalar_mul(nm[:], mx[:], -1.0)
                    ev = prt.tile([128, e], f32, tag="ev")
                    nc.scalar.activation(ev[:], ev_in, AF.Exp,
                                         bias=nm[:], scale=1.0)
                    t8 = prt.tile([128, 8], f32, tag="t8")
                    nc.vector.max(out=t8[:], in_=ev[:])
                    nc.vector.memset(t8[:, TOPK:], 0.0)
                    zap = prt.tile([128, e], f32, tag="zap")
                    nc.vector.match_replace(out=zap[:], in_to_replace=t8[:],
                                            in_values=ev[:], imm_value=0.0)
                    msk = prt.tile([128, e], f32, tag="msk")
                    nc.vector.tensor_sub(msk[:], ev[:], zap[:])
                    dn = prt.tile([128, 1], f32, tag="dn")
                    nc.vector.reduce_sum(out=dn[:], in_=msk[:], axis=X)
                    iv = prt.tile([128, 1], f32, tag="iv")
                    nc.vector.reciprocal(iv[:], dn[:])
                    nc.vector.tensor_scalar_mul(
                        route[:, tt * e:(tt + 1) * e], msk[:], iv[:])

                # route -> [expert, token] layout. The transpose input is
                # shifted by `le` so own-expert column le lands on partition
                # 0 of its block (compute APs need 32-aligned partition
                # bases, so reading rqs[1:2, :] later would be illegal).
                for le in range(EPC):
                    for tt in range(MT):
                        for b in range(4):
                            nc.vector.transpose(
                                out=rqs[0:32, le * t + tt * 128 + b * 32:
                                        le * t + tt * 128 + (b + 1) * 32],
                                in_=route[b * 32:(b + 1) * 32,
                                          tt * e + le:tt * e + le + 32])
                # ---- phase A, second slice, with the w2 prefetch DMAs
                # interleaved so they land just before phase B needs them
                emit_a_range(all_pairs[14:17])
                for kk in range(W2PRE):
                    nc.sync.dma_start(out=w2p0[:, kk * H:(kk + 1) * H],
                                      in_=w2d[0, kk])
                emit_a_range(all_pairs[17:])

                # broadcast each own-expert route row across all 128
                # partitions via a PE outer product (ones x row). Keeping
                # this off GpSimd matters: gpsimd ucode ops ahead of the
                # collectives were observed to stall the whole CC chain.
                with tc.tile_pool(name="psb2", bufs=1,
                                  space="PSUM") as ppb2:
                    for le in range(EPC):
                        pbc = ppb2.tile([128, t], f32, tag="pbc",
                                        name=f"pbc{le}")
                        for n0 in range(0, t, 512):
                            nc.tensor.matmul(
                                pbc[:, n0:n0 + 512],
                                lhsT=ones[0:1, :],
                                rhs=rqs[0:1, le * t + n0:le * t + n0 + 512],
                                start=True, stop=True)
                        nc.vector.tensor_copy(
                            out=rbc[:, le * t:(le + 1) * t], in_=pbc[:])
                for le in range(EPC):
                    for jj in range(NB):
                        blk = (le * NB + jj) * t
                        nc.vector.tensor_mul(
                            out=selw[0:BK, blk:blk + t],
                            in0=sels[0:BK, blk:blk + t],
                            in1=rbc[0:BK, le * t:(le + 1) * t])

            # second warm-up collective, data-gated on the end of phase A:
            # without it the CC path sits cold for ~160us and the first
            # real reduce-scatter ran 2-3x slower than the rest
            nc.sync.dma_start(out=wrm2_i[:, :],
                              in_=act[0:8, EPC * KF * C - 256:])
            nc.gpsimd.collective_compute(
                "ReduceScatter", Alu.add,
                replica_groups=[list(range(n_cores))],
                ins=[wrm2_i.ap().opt()],
                outs=[wrm2_o.ap().opt()],
            )

            # ---- phase B + weighted scatter + chunked reduce-scatter ----
            with (tc.tile_pool(name="w2p", bufs=EPC * KF - W2PRE) as pw2,
                  tc.tile_pool(name="yb", bufs=4) as pyb,
                  tc.tile_pool(name="so", bufs=2) as pso,
                  tc.tile_pool(name="psb", bufs=3, space="PSUM") as ppb,
                  tc.tile_pool(name="psc", bufs=2, space="PSUM") as ppc):
                w2sb = {}
                for le in range(EPC):
                    for kk in range(KF):
                        if le == 0 and kk < W2PRE:
                            w2sb[(le, kk)] = w2p0[:, kk * H:(kk + 1) * H]
                        else:
                            w2k = pw2.tile([128, H], bf16, tag="w2k")
                            nc.sync.dma_start(out=w2k[:], in_=w2d[le, kk])
                            w2sb[(le, kk)] = w2k[:]

                for jj in range(NB):
                    ybs = {}
                    for le in range(EPC):
                        py = [ppb.tile([128, 1024], f32, tag="py",
                                       name=f"py{jj}_{le}_{hh}")
                              for hh in range(2)]
                        for kk in range(KF):
                            lh = act[:, (le * KF + kk) * C + jj * BK:
                                     (le * KF + kk) * C + (jj + 1) * BK]
                            w2t_ = w2sb[(le, kk)]
                            for hh in range(2):
                                for q in range(2):
                                    n0 = q * 512
                                    nc.tensor.matmul(
                                        py[hh][0:BK, n0:n0 + 512],
                                        lhsT=lh,
                                        rhs=w2t_[:, hh * 1024 + n0:
                                                 hh * 1024 + n0 + 512],
                                        start=(kk == 0), stop=(kk == KF - 1))
                        yb = pyb.tile([128, H], bf16, tag="yb")
                        # ScalarE drains py so the DVE stays free for the
                        # scatter copies that gate the partial DMAs
                        for hh in range(2):
                            nc.scalar.activation(
                                yb[0:BK, hh * 1024:(hh + 1) * 1024],
                                py[hh][0:BK, :], AF.Copy)
                        ybs[le] = yb

                    for tt in (2 * jj, 2 * jj + 1):
                        so = pso.tile([128, H], bf16, tag="so",
                                      name=f"so{tt}")
                        for hq in range(4):
                            ps = ppc.tile([128, 512], f32, tag="ps",
                                          name=f"ps{tt}_{hq}")
                            for le in range(EPC):
                                blk = (le * NB + jj) * t
                                nc.tensor.matmul(
                                    ps[:],
                                    lhsT=selw[0:BK, blk + tt * 128:
                                              blk + (tt + 1) * 128],
                                    rhs=ybs[le][0:BK,
                                                hq * 512:(hq + 1) * 512],
                                    start=(le == 0), stop=(le == EPC - 1))
                            nc.vector.tensor_copy(
                                out=so[:, hq * 512:(hq + 1) * 512],
                                in_=ps[:])
                        # spread each partial store over 4 row-slice DMAs
                        # alternating between both DMA engines' ring sets:
                        # one ring moves ~40 GB/s, and backlogged partials
                        # delayed the reduce-scatter triggers 20-30 us
                        r0 = (tt % 2) * 128
                        for q in range(4):
                            eng = nc.sync if q % 2 == 0 else nc.scalar
                            eng.dma_start(
                                out=parts[jj][r0 + q * 32:
                                              r0 + (q + 1) * 32, :],
                                in_=so[q * 32:(q + 1) * 32, :])

                # collectives are emitted after the compute loop: each is
                # data-gated on its partial, so they still overlap B/scatter
                # of later chunks, but no sync edges land inside the PE/DVE
                # streams (in-loop emission measurably stalled both)
                for jj in range(NB):
                    nc.gpsimd.collective_compute(
                        "ReduceScatter", Alu.add,
                        replica_groups=[list(range(n_cores))],
                        ins=[parts[jj].ap().opt()],
                        outs=[rss[jj].ap().opt()],
                    )
                    nc.sync.dma_start(
                        out=out_sh[jj * shw:(jj + 1) * shw, :],
                        in_=rss[jj][:, :])

    nc.compile()
    return nc


def _route_sel(x, gate_w):
    """Host routing metadata: top-6 membership with a tie margin."""
    lg = x.astype(np.float64) @ gate_w.astype(np.float64).T
    lg -= lg.max(axis=1, keepdims=True)
    p = np.exp(lg)
    p /= p.sum(axis=1, keepdims=True)
    sp = -np.sort(-p, axis=1)
    thr = sp[:, TOPK - 1:TOPK] * (1.0 - MARGIN)
    return p >= thr


def fit_bk(sel):
    """Slot-bucket capacity: max (expert, tile-pair) count, rounded to 8."""
    tp = np.arange(T) // (2 * 128)
    mx = 0
    for ee in range(E):
        for jj in range(NB):
            mx = max(mx, int((sel[:, ee] & (tp == jj)).sum()))
    if mx > 128:
        raise ValueError(f"bucket overflow: {mx} > 128")
    return max(64, (mx + 7) // 8 * 8)


def prep_inputs(x, gate_w, wv1, w2, *_unused, BK=None):
    """Host-side shard/gather/cast/tile. Returns per-core input maps."""
    import ml_dtypes
    bf16 = ml_dtypes.bfloat16

    x = np.asarray(x, dtype=np.float32)
    gate_w = np.asarray(gate_w, dtype=np.float32)
    sel = _route_sel(x, gate_w)                       # [T, E] bool
    if BK is None:
        BK = fit_bk(sel)
    C = NB * BK
    tp = np.arange(T) // (2 * 128)                    # token-tile pair id

    xTf = np.ascontiguousarray(x.T).astype(np.float32)

    in_maps = []
    for c in range(NCORES):
        own = list(range(c * EPC, (c + 1) * EPC))
        rest = [i for i in range(E) if i not in own]
        perm = own + rest
        gp = gate_w[perm].T.astype(np.float32)        # [H, E]
        gwp = np.ascontiguousarray(
            gp.reshape(KH, 128, E).transpose(1, 0, 2).reshape(128, KH * E))

        toks = np.full((EPC, NB, BK), -1, dtype=np.int64)
        for le, ee in enumerate(own):
            for jj in range(NB):
                tt = np.nonzero(sel[:, ee] & (tp == jj))[0]
                if len(tt) > BK:
                    raise ValueError(
                        f"bucket overflow: expert {ee} pair {jj}: {len(tt)}")
                toks[le, jj, :len(tt)] = tt
        valid = toks >= 0
        tok0 = np.where(valid, toks, 0)

        xs = x[tok0.reshape(-1)].reshape(EPC, C, H) \
            * valid.reshape(EPC, C, 1)
        xgd = np.ascontiguousarray(
            xs.reshape(EPC, C, KH, 128).transpose(2, 3, 0, 1)
              .reshape(KH, 128, EPC * C)).astype(bf16)

        seldf = np.zeros((EPC, NB, BK, T), dtype=np.float32)
        il, ij, ii = np.nonzero(valid)
        seldf[il, ij, ii, toks[valid]] = 1.0
        seld = seldf.astype(bf16)

        wl = wv1[own]                                 # [EPC, 2F, H]
        wgd = np.ascontiguousarray(
            wl.reshape(EPC, MF2, 128, KH, 128)
              .transpose(0, 1, 4, 3, 2)
              .reshape(EPC, MF2, 128, KH * 128)).astype(bf16)

        w2l = w2[own]                                 # [EPC, H, F]
        w2d = np.ascontiguousarray(
            w2l.transpose(0, 2, 1).reshape(EPC, KF, 128, H)).astype(bf16)

        in_maps.append({
            "xT": xTf,
            "gwp": gwp,
            "xgd": xgd,
            "seld": seld,
            "wgd": wgd,
            "w2d": w2d,
        })
    return in_maps


def unshard(per_core_results):
    """Reassemble [T, H] from each core's stacked rs chunks."""
    shw = 2 * 128 // NCORES                           # 32 rows per chunk
    out = np.empty((T, H), dtype=np.float32)
    for c, res in enumerate(per_core_results):
        sh = np.asarray(res["out_sh"]).astype(np.float32)
        for jj in range(NB):
            base = jj * 2 * 128 + c * shw
            out[base:base + shw, :] = sh[jj * shw:(jj + 1) * shw, :]
    return out


def kernel(x, gate_w, wv1, w2, top_k):
    from concourse.bass_utils import run_bass_kernel_spmd

    assert int(top_k) == TOPK
    x = np.asarray(x, dtype=np.float32)
    gate_w = np.asarray(gate_w, dtype=np.float32)
    wv1 = np.asarray(wv1, dtype=np.float32)
    w2 = np.asarray(w2, dtype=np.float32)

    bk = fit_bk(_route_sel(x, gate_w))
    key = (T, H, F, E, NCORES, bk)
    if key not in _CACHE:
        _CACHE[key] = build_moe_nc(NCORES, BK=bk)
    nc = _CACHE[key]

    in_maps = prep_inputs(x, gate_w, wv1, w2, BK=bk)
    res = run_bass_kernel_spmd(nc, in_maps, list(range(NCORES)))
    return unshard([res.results[c] for c in range(NCORES)])
